# revision 32
# baseline (speedup 1.0000x reference)
"""Trainium2 Bass kernel for nn_MACEConvolutionLayer.

Strategy (8 NeuronCores, no collectives):
  - Edges sharded by destination-node range (1250 nodes/core), sorted and
    packed into 10 windows of 128 nodes x 1024 edge slots per core. Messages
    are segment-summed into node windows via host-precomputed one-hot
    scatter matmuls on the tensor engine.
  - Per-edge bilinear (radial features x embedded source scalars) and the
    per-node equivariant tensor products use a monomial scheme computed
    directly in transposed [uv, sample] layout: replicated factor tiles
    (built by DMA through a DRAM scratch roundtrip) are multiplied
    elementwise on DVE/GpSimd, and the tensor engine contracts the monomial
    chunks against packed combined weight matrices (Clebsch-Gordan x TP
    weights with channel mixing/combination folded in). This avoids all
    PE-transposes and PSUM evacuation copies of the previous scheme.
  - Output q|msg columns are interleaved per kappa-component so each
    (block, chunk) usually emits one contiguous column run.

Feature layout on device is kappa-major: col(l, i, u) = LOFF[l] + i*32 + u.
"""
import sys, os

sys.path.insert(0, '/opt/trn_rl_repo')

import numpy as np
import ml_dtypes

MUL = 32
DIMS = (1, 3, 5)
HID = 288
N_NODES = 10000
N_EDGES = 64000
RHID = 64
SQM = float(np.sqrt(MUL))
LOFF = [0, 32, 128]
SOFF = [0, 1, 4]
PATHS_FULL = [(0,0,0),(0,1,1),(0,2,2),(1,0,1),(1,1,0),(1,1,2),(1,2,1),(2,0,2),(2,1,1),(2,2,0),(2,2,2)]
O2_UVW = [(0,1,1),(0,2,2),(1,2,1)]
O2_UVU = [(0,0,0),(1,1,0),(1,1,2),(2,2,0),(2,2,2)]

N_CORES = 8
NODES_PER_CORE = 1250
WIN = 128
N_WIN = 10
ESLOT = 1024
E_PAD = N_WIN * ESLOT   # 10240
ET_PER_WIN = ESLOT // 128  # 8
BF = ml_dtypes.bfloat16

NCOMP = 9  # number of (l, i) components
MAX_JRUN = 2   # max J-run length per product op


def comp_ord(l, i):
    return LOFF[l] // 32 + i


COMP_L = [0, 1, 1, 1, 2, 2, 2, 2, 2]  # l of each component ordinal


def cg_np():
    s2, s3, s5, s6 = map(np.sqrt, (2.0, 3.0, 5.0, 6.0))
    B = np.zeros((5, 3, 3))
    B[0, 0, 1] = B[0, 1, 0] = 1 / s2
    B[1, 1, 2] = B[1, 2, 1] = 1 / s2
    B[2] = np.diag([-1.0, -1.0, 2.0]) / s6
    B[3, 0, 2] = B[3, 2, 0] = 1 / s2
    B[4] = np.diag([1.0, -1.0, 0.0]) / s2
    C = {}
    C[(0, 0, 0)] = np.ones((1, 1, 1))
    C[(0, 1, 1)] = (np.eye(3) / s3)[None]
    C[(1, 0, 1)] = np.transpose(C[(0, 1, 1)], (1, 0, 2))
    C[(0, 2, 2)] = (np.eye(5) / s5)[None]
    C[(2, 0, 2)] = np.transpose(C[(0, 2, 2)], (1, 0, 2))
    C[(1, 1, 0)] = (np.eye(3) / s3)[:, :, None]
    C[(1, 1, 2)] = np.transpose(B, (1, 2, 0)) / s5
    C[(1, 2, 1)] = np.transpose(B, (1, 0, 2)) / s5
    C[(2, 1, 1)] = B / s5
    C[(2, 2, 0)] = (np.eye(5) / s5)[:, :, None]
    T = np.einsum('aij,bjk,cki->abc', B, B, B)
    C[(2, 2, 2)] = T / np.linalg.norm(T)
    return C


CG = cg_np()
PATH_LIST_O2 = O2_UVW + O2_UVU


def support_pairs(path_ijk):
    d = {}
    for pi, (li, lj, lk) in enumerate(path_ijk):
        C = CG[(li, lj, lk)]
        for iloc in range(DIMS[li]):
            for jloc in range(DIMS[lj]):
                if np.any(np.abs(C[iloc, jloc, :]) > 1e-12):
                    d.setdefault(((li, iloc), (lj, jloc)), []).append((pi, iloc, jloc))
    return d


def build_mono_blocks_sym(path_ijk):
    d = support_pairs(path_ijk)
    blocks = {}
    for (I, J), lst in d.items():
        key = (min(I, J), max(I, J))
        swap = I > J
        for (pi, iloc, jloc) in lst:
            blocks.setdefault(key, []).append((pi, iloc, jloc, swap))
    return [(I, J, c) for (I, J), c in sorted(blocks.items())]


def build_mono_blocks(path_ijk):
    d = support_pairs(path_ijk)
    return [(I, J, [(pi, i, j, False) for (pi, i, j) in lst]) for (I, J), lst in sorted(d.items())]


def omega_for_block(path_ijk, weights, I, J, contribs, reg):
    """[1024 (u-major,v-fast), 576] interleaved outputs:
    col(g_out, reg, w) = g_out*64 + reg*32 + w."""
    Om = np.zeros((MUL * MUL, 2 * HID))
    for (pi, iloc, jloc, swap) in contribs:
        li, lj, lk = path_ijk[pi]
        W = weights[pi]
        C = CG[(li, lj, lk)]
        for kap in range(DIMS[lk]):
            c = C[iloc, jloc, kap]
            if abs(c) < 1e-12:
                continue
            gk = comp_ord(lk, kap)
            c0 = gk * 64 + reg * 32
            Wm = W if not swap else np.transpose(W, (1, 0, 2))
            Om[:, c0:c0 + 32] += c * Wm.reshape(MUL * MUL, MUL)
    return Om


# ---------------------------------------------------------------------------
# static plan
# ---------------------------------------------------------------------------

class Plan:
    pass


def _emissions(mask):
    """mask: [1024, 576] bool. Returns per kc: list of (c0, c1) col runs
    (gaptol 0 at 32-col-slot granularity, split at 512-wide)."""
    out = []
    for kc in range(8):
        sub = mask[kc * 128:(kc + 1) * 128]
        slots = [s for s in range(18) if np.any(sub[:, s * 32:(s + 1) * 32])]
        runs = []
        for s in slots:
            if runs and s == runs[-1][1]:
                runs[-1][1] = s + 1
            else:
                runs.append([s, s + 1])
        emis = []
        for (a, b) in runs:
            while (b - a) * 32 > 512:
                emis.append((a * 32, a * 32 + 512))
                a += 16
            emis.append((a * 32, b * 32))
        out.append(emis)
    return out


def build_plan():
    p = Plan()
    aa_blocks = build_mono_blocks_sym(PATHS_FULL + PATH_LIST_O2)
    qa_blocks = build_mono_blocks(PATHS_FULL)
    n3a = len(PATHS_FULL)
    ones_a = [np.ones((MUL, MUL, MUL)) for _ in PATHS_FULL]
    ones_o2 = [np.ones((MUL, MUL, MUL)) for _ in PATH_LIST_O2]

    p.aa = []
    for (I, J, contribs) in aa_blocks:
        cq = [(pi, i, j, s) for (pi, i, j, s) in contribs if pi < n3a]
        cm = [(pi - n3a, i, j, s) for (pi, i, j, s) in contribs if pi >= n3a]
        mask = np.zeros((1024, 576), bool)
        if cq:
            mask |= omega_for_block(PATHS_FULL, ones_a, I, J, cq, 0) != 0
        if cm:
            mask |= omega_for_block(PATH_LIST_O2, ones_o2, I, J, cm, 1) != 0
        p.aa.append((I, J, cq, cm, _emissions(mask)))
    p.qa = []
    for (I, J, contribs) in qa_blocks:
        mask = omega_for_block(PATHS_FULL, ones_a, I, J, contribs, 1) != 0
        p.qa.append((I, J, contribs, _emissions(mask)))

    # omega column offsets
    off = 0
    p.aa_emi = []
    for (I, J, cq, cm, em) in p.aa:
        bk = []
        for kc in range(8):
            lst = []
            for (c0, c1) in em[kc]:
                lst.append((c0, c1, off))
                off += c1 - c0
            bk.append(lst)
        p.aa_emi.append(bk)
    p.qa_emi = []
    for (I, J, contribs, em) in p.qa:
        bk = []
        for kc in range(8):
            lst = []
            for (c0, c1) in em[kc]:
                lst.append((c0, c1, off))
                off += c1 - c0
            bk.append(lst)
        p.qa_emi.append(bk)
    p.totc = off
    p.n_emi = sum(len(l) for bk in p.aa_emi + p.qa_emi for l in bk)

    # J-run groups for product ops: consecutive blocks with same I and
    # consecutive J ordinals, capped at MAX_JRUN
    def groups(blocks):
        gs = []
        for bi, blk in enumerate(blocks):
            I, J = blk[0], blk[1]
            gI = comp_ord(*I); gJ = comp_ord(*J)
            if (gs and gs[-1][0] == gI and gs[-1][1] + gs[-1][2] == gJ
                    and gs[-1][2] < MAX_JRUN):
                gs[-1][2] += 1
            else:
                gs.append([gI, gJ, 1, bi])
        return [(gI, gJ, n, b0) for (gI, gJ, n, b0) in gs]

    p.aa_groups = groups(p.aa)
    p.qa_groups = groups(p.qa)
    return p


def pack_omega(plan, Wfold):
    W3a = Wfold['o3a_w']; Wo2 = Wfold['o2_w']; W3b = Wfold['o3b_w']
    om = np.zeros((128, plan.totc), np.float32)
    for bi, (I, J, cq, cm, em) in enumerate(plan.aa):
        Om = np.zeros((MUL * MUL, 2 * HID))
        if cq:
            Om += omega_for_block(PATHS_FULL, W3a, I, J, cq, 0)
        if cm:
            Om += omega_for_block(PATH_LIST_O2, Wo2, I, J, cm, 1)
        for kc in range(8):
            for (c0, c1, off) in plan.aa_emi[bi][kc]:
                om[:, off:off + (c1 - c0)] = Om[kc * 128:(kc + 1) * 128, c0:c1]
    for bi, (I, J, contribs, em) in enumerate(plan.qa):
        Om = omega_for_block(PATHS_FULL, W3b, I, J, contribs, 1)
        for kc in range(8):
            for (c0, c1, off) in plan.qa_emi[bi][kc]:
                om[:, off:off + (c1 - c0)] = Om[kc * 128:(kc + 1) * 128, c0:c1]
    return om.astype(BF)


def fold_weights(inp):
    f8 = np.float64
    mix_w = inp['mix_w'].astype(f8); comb_w = inp['comb_w'].astype(f8)
    M = np.einsum('olux,olxw->oluw', mix_w, comb_w) / MUL
    W1eff = np.einsum('lux,lxw->luw', inp['lin_o1'].astype(f8), M[0]) / SQM
    o2_w = []
    for pp, (i, j, k) in enumerate(O2_UVW):
        o2_w.append(np.einsum('uvx,xw->uvw', inp['o2_uvw'][pp].astype(f8) / MUL, M[1][k]))
    for pp, (i, j, k) in enumerate(O2_UVU):
        o2_w.append(np.einsum('uv,uw->uvw', inp['o2_uvu'][pp].astype(f8), M[1][k]) / SQM)
    o3a_w = [inp['o3a_uvw'][pp].astype(f8) / MUL for pp in range(len(PATHS_FULL))]
    o3b_w = [np.einsum('uvx,xw->uvw', inp['o3b_uvw'][pp].astype(f8) / MUL, M[2][k])
             for pp, (i, j, k) in enumerate(PATHS_FULL)]
    aw = inp['a_w'].astype(f8).reshape(RHID, 3, MUL, MUL)
    ab = inp['a_b'].astype(f8).reshape(3, MUL, MUL)
    scale = np.array([1.0 / np.sqrt(d) for d in DIMS]) / SQM
    aw = aw * scale[None, :, None, None]
    ab = ab * scale[:, None, None]
    A2 = np.transpose(aw, (0, 2, 1, 3)).reshape(RHID * MUL, 3 * MUL)
    B2 = np.transpose(ab, (1, 0, 2)).reshape(MUL, 3 * MUL)
    # omc1: [32, 3*32]: per-l 32x32 order-1 linear (same for all i of that l)
    omc1 = np.zeros((32, 96))
    for l in range(3):
        omc1[:, l * 32:(l + 1) * 32] = W1eff[l]
    return dict(
        o3a_w=o3a_w, o2_w=o2_w, o3b_w=o3b_w,
        omc1=omc1, omself=inp['self_w'].astype(f8) / SQM,
        emb=inp['emb_w'].astype(f8) / SQM,
        A2=A2, B2=B2,
        r_w1=inp['r_w1'].astype(np.float32), r_b1=inp['r_b1'].astype(np.float32),
        r_w2=inp['r_w2'].astype(np.float32), r_b2=inp['r_b2'].astype(np.float32),
        r_w3=inp['r_w3'].astype(np.float32), r_b3=inp['r_b3'].astype(np.float32),
    )


def pack_edges(inp):
    src = np.asarray(inp['edge_index'][0]).astype(np.int64)
    dst = np.asarray(inp['edge_index'][1]).astype(np.int64)
    sh = np.asarray(inp['edge_sh'], dtype=np.float32)
    rad = np.asarray(inp['edge_radial_embedding'], dtype=np.float32)
    attr = np.asarray(inp['edge_attr'], dtype=np.float32)
    nf = np.asarray(inp['node_features'], dtype=np.float32)
    cnt = np.bincount(dst, minlength=N_NODES).astype(np.float32)
    rec_all = 1.0 / np.maximum(cnt, 1.0)
    order = np.argsort(dst, kind='stable')
    dst_s = dst[order]
    cores = []
    for c in range(N_CORES):
        lo = c * NODES_PER_CORE
        rinT = np.zeros((24, E_PAD), np.float32)
        nfsT = np.zeros((MUL, E_PAD), np.float32)
        sh9 = np.zeros((E_PAD, 9), np.float32)
        S = np.zeros((E_PAD, 128), BF)
        for w in range(N_WIN):
            nlo = lo + w * WIN
            nhi = min(lo + (w + 1) * WIN, lo + NODES_PER_CORE)
            a = np.searchsorted(dst_s, nlo); b = np.searchsorted(dst_s, nhi)
            idx = order[a:b]
            n = b - a
            assert n <= ESLOT, f"window overflow {n}"
            s = w * ESLOT
            rinT[:8, s:s + n] = rad[idx].T
            rinT[8:, s:s + n] = attr[idx].T
            nfsT[:, s:s + n] = nf[src[idx]].T
            sh9[s:s + n, :] = sh[idx]
            S[s + np.arange(n), (dst[idx] - nlo)] = BF(1.0)
        nfT = np.zeros((MUL, N_WIN * WIN), BF)
        nfT[:, :NODES_PER_CORE] = nf[lo:lo + NODES_PER_CORE].T.astype(BF)
        rec = np.ones((N_WIN * WIN, 1), np.float32)
        rec[:NODES_PER_CORE, 0] = rec_all[lo:lo + NODES_PER_CORE]
        cores.append(dict(rinT=rinT, nfsT=nfsT, sh9=sh9, S=S, nfT=nfT, rec=rec))
    return cores


def ref_from_kap(x_kap):
    out = np.empty_like(x_kap)
    for l, d in enumerate(DIMS):
        blk = x_kap[:, LOFF[l]:LOFF[l] + 32 * d].reshape(-1, d, 32)
        out[:, LOFF[l]:LOFF[l] + 32 * d] = np.transpose(blk, (0, 2, 1)).reshape(-1, 32 * d)
    return out


# ---------------------------------------------------------------------------
# device kernel
# ---------------------------------------------------------------------------

_NC_CACHE = {}
LAST_RESULT = None

# fraction of product work sent to gpsimd (tuned from profiles)
GP_ELEM_NS = 99.0e-3   # us per free-elem (effectively disable gpsimd)
VE_ELEM_NS = 0.52e-3
GP_OP_OH = 0.25
VE_OP_OH = 0.08


def build_nc(plan):
    import concourse.bass as bass
    import concourse.bacc as bacc
    import concourse.mybir as mybir
    import concourse.tile as tile

    f32 = mybir.dt.float32
    bf16 = mybir.dt.bfloat16
    AL = mybir.AluOpType
    AF = mybir.ActivationFunctionType

    nc = bacc.Bacc(None)
    P = 128

    # ---- dram parameters
    rinT_d = nc.declare_dram_parameter("rinT", [24, E_PAD], f32, isOutput=False)
    nfsT_d = nc.declare_dram_parameter("nfsT", [32, E_PAD], f32, isOutput=False)
    sh9_d = nc.declare_dram_parameter("sh9", [E_PAD, 9], f32, isOutput=False)
    S_d = nc.declare_dram_parameter("S", [E_PAD, 128], bf16, isOutput=False)
    nfT_d = nc.declare_dram_parameter("nfT", [32, N_WIN * WIN], bf16, isOutput=False)
    rec_d = nc.declare_dram_parameter("rec", [N_WIN * WIN, 1], f32, isOutput=False)
    omega_d = nc.declare_dram_parameter("omega", [P, plan.totc], bf16, isOutput=False)
    a2_d = nc.declare_dram_parameter("a2", [P, 16 * 96], bf16, isOutput=False)
    b2_d = nc.declare_dram_parameter("b2", [32, 96], bf16, isOutput=False)
    omc1_d = nc.declare_dram_parameter("omc1", [32, 96], bf16, isOutput=False)
    omself_d = nc.declare_dram_parameter("omself", [32, 32], bf16, isOutput=False)
    rw1_d = nc.declare_dram_parameter("rw1", [24, 64], f32, isOutput=False)
    rw2_d = nc.declare_dram_parameter("rw2", [64, 64], f32, isOutput=False)
    rw3_d = nc.declare_dram_parameter("rw3", [64, 64], f32, isOutput=False)
    rb1_d = nc.declare_dram_parameter("rb1", [64, 1], f32, isOutput=False)
    rb2_d = nc.declare_dram_parameter("rb2", [64, 1], f32, isOutput=False)
    emb_d = nc.declare_dram_parameter("emb", [32, 32], f32, isOutput=False)
    identb_d = nc.declare_dram_parameter("identb", [P, P], bf16, isOutput=False)
    selfull_d = nc.declare_dram_parameter("selfull", [P, 1024], bf16, isOutput=False)
    selr_d = nc.declare_dram_parameter("selr", [64, 2048], bf16, isOutput=False)
    repfull_d = nc.declare_dram_parameter("repfull", [P, P], bf16, isOutput=False)
    zer_d = nc.declare_dram_parameter("zer", [1, P], bf16, isOutput=False)
    zer2_d = nc.declare_dram_parameter("zer2", [1, 2 * HID], bf16, isOutput=False)
    out_d = nc.declare_dram_parameter("out", [N_WIN * WIN, HID], f32, isOutput=True)

    # engine schedule for product ops: greedy balance vector vs gpsimd
    def make_sched():
        ops = []
        for gi, (gI, gJ, nJ, b0) in enumerate(plan.aa_groups):
            ops.append(('aa', gi, nJ * 1024))
        for gi, (gI, gJ, nJ, b0) in enumerate(plan.qa_groups):
            ops.append(('qa', gi, nJ * 1024))
        for q in range(4):
            ops.append(('edge', q, 4096))
        v_t, g_t = 1.5, 0.0   # vector pre-loaded with msgs/evac budget
        sched = {}
        for (kind, idx, wdt) in ops:
            vc = wdt * VE_ELEM_NS + VE_OP_OH
            gc = wdt * GP_ELEM_NS + GP_OP_OH
            if g_t + gc < v_t + vc:
                sched[(kind, idx)] = 'gpsimd'; g_t += gc
            else:
                sched[(kind, idx)] = 'vector'; v_t += vc
        return sched

    sched = make_sched()

    from contextlib import ExitStack
    with tile.TileContext(nc) as tc, ExitStack() as es:
        cst = es.enter_context(tc.tile_pool(name="cst", bufs=1))
        sb2 = es.enter_context(tc.tile_pool(name="sb2", bufs=2))
        sb3 = es.enter_context(tc.tile_pool(name="sb3", bufs=2))
        uu_pool = es.enter_context(tc.tile_pool(name="uu", bufs=1))
        pt_pool = es.enter_context(tc.tile_pool(name="pt", bufs=8))
        ed_pool = es.enter_context(tc.tile_pool(name="ed", bufs=1))
        sb1 = es.enter_context(tc.tile_pool(name="sb1", bufs=1))
        ps_wps = es.enter_context(tc.tile_pool(name="pswps", bufs=1, space="PSUM"))
        ps_uub = es.enter_context(tc.tile_pool(name="psuub", bufs=2, space="PSUM"))
        ps_qm = es.enter_context(tc.tile_pool(name="psqm", bufs=1, space="PSUM"))
        ps_tp = es.enter_context(tc.tile_pool(name="pstp", bufs=1, space="PSUM"))
        ps_mlp = es.enter_context(tc.tile_pool(name="psmlp", bufs=1, space="PSUM"))
        ps_mx = es.enter_context(tc.tile_pool(name="psmx", bufs=1, space="PSUM"))

        # ---- constants
        omega = cst.tile([P, plan.totc], bf16)
        nc.sync.dma_start(out=omega[:], in_=omega_d[:])
        a2 = cst.tile([P, 16 * 96], bf16)
        nc.sync.dma_start(out=a2[:], in_=a2_d[:])
        b2 = cst.tile([32, 96], bf16); nc.sync.dma_start(out=b2[:], in_=b2_d[:])
        omc1 = cst.tile([32, 96], bf16); nc.sync.dma_start(out=omc1[:], in_=omc1_d[:])
        omself = cst.tile([32, 32], bf16); nc.sync.dma_start(out=omself[:], in_=omself_d[:])
        rw1 = cst.tile([24, 64], f32); nc.sync.dma_start(out=rw1[:], in_=rw1_d[:])
        rw2 = cst.tile([64, 64], f32); nc.sync.dma_start(out=rw2[:], in_=rw2_d[:])
        rw3 = cst.tile([64, 64], f32); nc.sync.dma_start(out=rw3[:], in_=rw3_d[:])
        rb1 = cst.tile([64, 1], f32); nc.sync.dma_start(out=rb1[:], in_=rb1_d[:])
        rb2 = cst.tile([64, 1], f32); nc.sync.dma_start(out=rb2[:], in_=rb2_d[:])
        emb = cst.tile([32, 32], f32); nc.sync.dma_start(out=emb[:], in_=emb_d[:])
        identb = cst.tile([P, P], bf16); nc.sync.dma_start(out=identb[:], in_=identb_d[:])
        selfull = cst.tile([P, 1024], bf16); nc.sync.dma_start(out=selfull[:], in_=selfull_d[:])
        selr = cst.tile([64, 2048], bf16); nc.sync.dma_start(out=selr[:], in_=selr_d[:])
        repfull = cst.tile([P, P], bf16); nc.sync.dma_start(out=repfull[:], in_=repfull_d[:])
        zer = cst.tile([1, P], bf16); nc.sync.dma_start(out=zer[:], in_=zer_d[:])
        zer2 = cst.tile([1, 2 * HID], bf16); nc.sync.dma_start(out=zer2[:], in_=zer2_d[:])
        nfT = cst.tile([32, N_WIN * WIN], bf16)
        nc.sync.dma_start(out=nfT[:], in_=nfT_d[:])

        def transpose3(x_bf, tag):
            """x_bf [128, 288] bf16 -> aT sbuf [128, 384] (chunk-major)."""
            tp = ps_tp.tile([P, 384], bf16, space="PSUM", tag="tp")
            nc.tensor.transpose(out=tp[:, 0:P], in_=x_bf[:, 0:P], identity=identb[:])
            nc.tensor.transpose(out=tp[:, P:2 * P], in_=x_bf[:, P:2 * P], identity=identb[:])
            nc.tensor.transpose(out=tp[0:32, 2 * P:3 * P], in_=x_bf[:, 2 * P:HID], identity=identb[:])
            xt = sb2.tile([P, 384], bf16, tag=tag + "sb")
            nc.scalar.copy(out=xt[:, 0:2 * P], in_=tp[:, 0:2 * P])
            nc.scalar.copy(out=xt[0:32, 2 * P:3 * P], in_=tp[0:32, 2 * P:3 * P])
            return xt

        NCC = [3, 2, 2, 2]   # comps per partition-row-group b: g = 4*cc + b <= 8

        def build_uu(aT, uu_tile, ev):
            """uu[32*u4+v, (g,kc,n)] = aT-val[f=32g+4kc+u4, node n] via SEL matmuls."""
            for b in range(4):
                ncc = NCC[b]
                for kc in range(8):
                    up = ps_uub.tile([P, 512], f32, space="PSUM", tag="uub")
                    nc.tensor.matmul(out=up[:, :ncc * P],
                                     lhsT=selfull[32 * b:32 * (b + 1), kc * P:(kc + 1) * P],
                                     rhs=aT[32 * b:32 * (b + 1), :ncc * P],
                                     start=True, stop=True, tile_position=(32 * b, 0))
                    dst = uu_tile[:].rearrange("p (g k n) -> p g k n", k=8, n=P)[:, b::4, kc, :]
                    src_ = up[:, :ncc * P].rearrange("p (c n) -> p c n", n=P)
                    nc.scalar.copy(out=dst, in_=src_)

        def build_v8(aT, v8_tile, ev):
            """v8[32*b+v, (g,n)] = aT-val[f=32g+v, node n] (mod-32 replication)."""
            for b in range(4):
                ncc = NCC[b]
                up = ps_uub.tile([P, 512], f32, space="PSUM", tag="uub")
                nc.tensor.matmul(out=up[:, :ncc * P],
                                 lhsT=repfull[32 * b:32 * (b + 1), :],
                                 rhs=aT[32 * b:32 * (b + 1), :ncc * P],
                                 start=True, stop=True, tile_position=(32 * b, 0))
                dst = v8_tile[:].rearrange("p (g n) -> p g n", n=P)[:, b::4, :]
                src_ = up[:, :ncc * P].rearrange("p (c n) -> p c n", n=P)
                nc.scalar.copy(out=dst, in_=src_)

        def emit_product_group(gi, groups, blocks, emi, uu, v8, qm, kind):
            (gI, gJ, nJ, b0) = groups[gi]
            wdt = nJ * 1024
            PT = pt_pool.tile([P, MAX_JRUN * 1024], bf16, tag="PT")
            eng = nc.gpsimd if sched[(kind, gi)] == 'gpsimd' else nc.vector
            eng.tensor_tensor(
                out=PT[:, :wdt].rearrange("p (j k n) -> p j k n", k=8, n=P),
                in0=uu[:, gI * 1024:(gI + 1) * 1024]
                    .rearrange("p (k n) -> p k n", n=P)[:, None, :, :]
                    .broadcast_to([P, nJ, 8, P]),
                in1=v8[:, gJ * P:(gJ + nJ) * P]
                    .rearrange("p (j n) -> p j n", n=P)[:, :, None, :]
                    .broadcast_to([P, nJ, 8, P]),
                op=AL.mult)
            for jl in range(nJ):
                bi = b0 + jl
                for kc in range(8):
                    for (c0, c1, off) in emi[bi][kc]:
                        nc.tensor.matmul(out=qm[:, c0:c1],
                                         lhsT=PT[:, jl * 1024 + kc * P: jl * 1024 + (kc + 1) * P],
                                         rhs=omega[:, off:off + (c1 - c0)],
                                         start=False, stop=False,
                                         skip_group_check=True)
        # per-window state for the software pipeline
        st = {}

        def emit_mlp_half(w, h):
            e0 = w * ESLOT
            if h == 0:
                rfT_t = sb1.tile([64, ESLOT], bf16, tag="rfT")
                hT_t = sb1.tile([32, ESLOT], bf16, tag="hT")
                vh_t = sb1.tile([P, ESLOT], bf16, tag="vh")
                wps_t = ps_wps.tile([P, HID], f32, space="PSUM", tag="wps")
                st[w] = dict(rfT=rfT_t, hT=hT_t, vh=vh_t, wps=wps_t, ev=[0])
            S = st[w]
            s = e0 + h * 512
            rin_h = sb2.tile([24, 512], f32, tag="rin")
            nc.sync.dma_start(out=rin_h[:], in_=rinT_d[:, s:s + 512])
            nfs_h = sb2.tile([32, 512], f32, tag="nfs")
            nc.sync.dma_start(out=nfs_h[:], in_=nfsT_d[:, s:s + 512])
            l1p = ps_mlp.tile([64, 512], f32, space="PSUM", tag="mlp")
            nc.tensor.matmul(out=l1p[:], lhsT=rw1[:], rhs=rin_h[:], start=True, stop=True)
            f1 = sb2.tile([64, 512], f32, tag="f")
            nc.scalar.activation(out=f1[:], in_=l1p[:], func=AF.Silu, bias=rb1[:], scale=1.0)
            l2p = ps_mlp.tile([64, 512], f32, space="PSUM", tag="mlp")
            nc.tensor.matmul(out=l2p[:], lhsT=rw2[:], rhs=f1[:], start=True, stop=True)
            f2 = sb2.tile([64, 512], f32, tag="f")
            nc.scalar.activation(out=f2[:], in_=l2p[:], func=AF.Silu, bias=rb2[:], scale=1.0)
            rfp = ps_mlp.tile([64, 512], f32, space="PSUM", tag="mlp")
            nc.tensor.matmul(out=rfp[:], lhsT=rw3[:], rhs=f2[:], start=True, stop=True)
            nc.scalar.copy(out=S['rfT'][:, h * 512:(h + 1) * 512], in_=rfp[:])
            hp = ps_mlp.tile([32, 512], f32, space="PSUM", tag="mlp")
            nc.tensor.matmul(out=hp[:], lhsT=emb[:], rhs=nfs_h[:], start=True, stop=True)
            nc.scalar.copy(out=S['hT'][:, h * 512:(h + 1) * 512], in_=hp[:])
            for b in range(4):
                nc.scalar.dma_start(out=S['vh'][32 * b:32 * (b + 1), h * 512:(h + 1) * 512],
                                    in_=S['hT'][:, h * 512:(h + 1) * 512])

        def emit_edge_quarter(w, q):
            e0 = w * ESLOT
            S = st[w]
            rfT, hT, vh, wps = S['rfT'], S['hT'], S['vh'], S['wps']
            uurf = ed_pool.tile([P, 4096], bf16, tag="uurf")
            for cp in range(8):
                up = ps_uub.tile([P, 512], f32, space="PSUM", tag="uub")
                for ci in range(2):
                    c = cp * 2 + ci
                    nc.tensor.matmul(out=up[:, ci * 256:(ci + 1) * 256],
                                     lhsT=selr[:, c * P:(c + 1) * P],
                                     rhs=rfT[:, q * 256:(q + 1) * 256],
                                     start=True, stop=True)
                nc.scalar.copy(out=uurf[:, cp * 512:(cp + 1) * 512], in_=up[:])
            mT = ed_pool.tile([P, 4096], bf16, tag="mT")
            eng = nc.gpsimd if sched[('edge', q)] == 'gpsimd' else nc.vector
            eng.tensor_tensor(
                out=mT[:].rearrange("p (c e) -> p c e", e=256),
                in0=uurf[:].rearrange("p (c e) -> p c e", e=256),
                in1=vh[:, q * 256:(q + 1) * 256][:, None, :].broadcast_to([P, 16, 256]),
                op=AL.mult)
            for tt in range(2):
                t = q * 2 + tt
                et = e0 + t * P
                mxp = ps_mx.tile([P, 96], f32, space="PSUM", tag="mx")
                for c in range(16):
                    nc.tensor.matmul(out=mxp[:], lhsT=mT[:, c * 256 + tt * P:c * 256 + (tt + 1) * P],
                                     rhs=a2[:, c * 96:(c + 1) * 96],
                                     start=(c == 0), stop=False)
                nc.tensor.matmul(out=mxp[:], lhsT=hT[:, t * P:(t + 1) * P], rhs=b2[:],
                                 start=False, stop=True)
                sh_t = sb3.tile([P, 9], f32, tag="sht")
                nc.sync.dma_start(out=sh_t[:], in_=sh9_d[et:et + P, :])
                msgs = sb2.tile([P, HID], bf16, tag="msgs")
                for l, d in enumerate(DIMS):
                    nc.vector.tensor_tensor(
                        out=msgs[:, LOFF[l]:LOFF[l] + 32 * d].rearrange("p (i u) -> p i u", u=32),
                        in0=sh_t[:, SOFF[l]:SOFF[l] + d][:, :, None].broadcast_to([P, d, 32]),
                        in1=mxp[:, l * 32:(l + 1) * 32][:, None, :].broadcast_to([P, d, 32]),
                        op=AL.mult)
                S_t = sb3.tile([P, P], bf16, tag="St")
                nc.sync.dma_start(out=S_t[:], in_=S_d[et:et + P, :])
                nc.tensor.matmul(out=wps[:], lhsT=S_t[:], rhs=msgs[:],
                                 start=(t == 0), stop=(t == ET_PER_WIN - 1))

        def emit_node_prefix(w):
            S = st[w]
            rec_t = sb2.tile([P, 1], f32, tag="rec")
            nc.sync.dma_start(out=rec_t[:], in_=rec_d[w * P:(w + 1) * P, :])
            a_bf = sb2.tile([P, HID], bf16, tag="abf")
            nc.vector.tensor_scalar_mul(out=a_bf[:], in0=S['wps'][:], scalar1=rec_t[:])
            aT = transpose3(a_bf, "at")
            uu = uu_pool.tile([P, NCOMP * 1024], bf16, tag="uu")
            build_uu(aT, uu, S['ev'])
            v8 = sb2.tile([P, NCOMP * P], bf16, tag="v8")
            build_v8(aT, v8, S['ev'])
            qm = ps_qm.tile([P, 2 * HID], f32, space="PSUM", tag="qm")
            nc.tensor.matmul(out=qm[:, 0:512], lhsT=zer[:], rhs=zer2[:, 0:512], start=True, stop=False, skip_group_check=True)
            nc.tensor.matmul(out=qm[:, 512:576], lhsT=zer[:], rhs=zer2[:, 512:576], start=True, stop=False, skip_group_check=True)
            S.update(uu=uu, v8=v8, qm=qm)

        def emit_node_qmid(w):
            S = st[w]
            q_bf = sb2.tile([P, HID], bf16, tag="qbf")
            nc.scalar.copy(
                out=q_bf[:].rearrange("p (g c) -> p g c", c=32),
                in_=S['qm'][:].rearrange("p (g t c) -> p g t c", t=2, c=32)[:, :, 0, :])
            qT = transpose3(q_bf, "qt")
            uuq = uu_pool.tile([P, NCOMP * 1024], bf16, tag="uu")
            build_uu(qT, uuq, S['ev'])
            S.update(uuq=uuq)

        def emit_node_suffix(w):
            S = st[w]
            qm, v8 = S['qm'], S['v8']
            for g in range(NCOMP):
                l = COMP_L[g]
                nc.tensor.matmul(out=qm[:, g * 64 + 32:g * 64 + 64],
                                 lhsT=v8[0:32, g * P:(g + 1) * P],
                                 rhs=omc1[:, l * 32:(l + 1) * 32],
                                 start=False, stop=False, skip_group_check=True)
            nc.tensor.matmul(out=qm[:, 32:64], lhsT=nfT[:, w * P:(w + 1) * P],
                             rhs=omself[:], start=False, stop=True,
                             skip_group_check=True)
            out_sb = sb1.tile([P, HID], f32, tag="outsb")
            nc.scalar.copy(
                out=out_sb[:].rearrange("p (g c) -> p g c", c=32),
                in_=qm[:].rearrange("p (g t c) -> p g t c", t=2, c=32)[:, :, 1, :])
            nc.sync.dma_start(out=out_d[w * P:(w + 1) * P, :], in_=out_sb[:])
            del st[w]

        # ---------------- software-pipelined main loop ----------------
        # edge phase of window w+1 is interleaved into node phase of window w
        # so the PE queue always has ready work (keeps HAM warm).
        
        def node_events(w):
            ev = []
            ev.append(lambda w=w: emit_node_prefix(w))
            for gi in range(len(plan.aa_groups)):
                ev.append(lambda w=w, gi=gi: emit_product_group(gi, plan.aa_groups, plan.aa, plan.aa_emi, st[w]['uu'], st[w]['v8'], st[w]['qm'], 'aa'))
            ev.append(lambda w=w: emit_node_qmid(w))
            for gi in range(len(plan.qa_groups)):
                ev.append(lambda w=w, gi=gi: emit_product_group(gi, plan.qa_groups, plan.qa, plan.qa_emi, st[w]['uuq'], st[w]['v8'], st[w]['qm'], 'qa'))
            ev.append(lambda w=w: emit_node_suffix(w))
            return ev

        def edge_events(w):
            ev = [lambda w=w: emit_mlp_half(w, 0),
                  lambda w=w: emit_edge_quarter(w, 0),
                  lambda w=w: emit_edge_quarter(w, 1),
                  lambda w=w: emit_mlp_half(w, 1),
                  lambda w=w: emit_edge_quarter(w, 2),
                  lambda w=w: emit_edge_quarter(w, 3)]
            return ev

        # prologue: edge phase of window 0 runs alone
        for f in edge_events(0):
            f()
        for w in range(N_WIN):
            for f in node_events(w):
                f()
            if w + 1 < N_WIN:
                for f in edge_events(w + 1):
                    f()

    nc.finalize()
    return nc


def _get_nc(plan):
    if 'nc' not in _NC_CACHE:
        _NC_CACHE['nc'] = build_nc(plan)
    return _NC_CACHE['nc']


def kernel(**inputs):
    global LAST_RESULT
    from concourse.bass_utils import run_bass_kernel_spmd

    inp = {k: np.asarray(v) for k, v in inputs.items()}
    plan = build_plan()
    W = fold_weights(inp)
    om = pack_omega(plan, W)

    A2 = W['A2'].astype(np.float32)
    a2p = np.zeros((128, 16 * 96), np.float32)
    for c in range(16):
        a2p[:, c * 96:(c + 1) * 96] = A2[c * 128:(c + 1) * 128, :]
    # fold r_b3 into B2 (rf = f2 @ rw3; +b3 contribution is linear in h)
    B2 = W['B2'].astype(np.float64).copy()
    b3 = inp['r_b3'].astype(np.float64)
    for u in range(32):
        B2[u, :] += b3 @ A2[np.arange(RHID) * 32 + u, :].astype(np.float64)

    identb = np.eye(128, dtype=np.float32).astype(BF)
    self = None
    selfull = np.zeros((128, 1024), np.float32)
    for p in range(128):
        for kc in range(8):
            u4 = p % 32 - 4 * kc
            if 0 <= u4 < 4:
                selfull[p, kc * 128 + u4 * 32:kc * 128 + (u4 + 1) * 32] = 1.0
    selr = np.zeros((64, 2048), np.float32)
    for q in range(64):
        c, r4 = divmod(q, 4)
        selr[q, c * 128 + r4 * 32:c * 128 + (r4 + 1) * 32] = 1.0
    repfull = np.zeros((128, 128), np.float32)
    for p in range(128):
        for i in range(128):
            if i % 32 == p % 32:
                repfull[p, i] = 1.0

    shared = dict(
        omega=om,
        a2=a2p.astype(BF), b2=B2.astype(np.float32).astype(BF),
        omc1=W['omc1'].astype(np.float32).astype(BF),
        omself=W['omself'].astype(np.float32).astype(BF),
        rw1=W['r_w1'], rw2=W['r_w2'], rw3=W['r_w3'],
        rb1=W['r_b1'].reshape(64, 1), rb2=W['r_b2'].reshape(64, 1),
        emb=W['emb'].astype(np.float32),
        identb=identb,
        selfull=selfull.astype(BF), selr=selr.astype(BF), repfull=repfull.astype(BF),
        zer=np.zeros((1, 128), BF), zer2=np.zeros((1, 2 * HID), BF),
    )
    cores = pack_edges(inp)
    in_maps = []
    for c in range(N_CORES):
        m = dict(shared)
        m.update(rinT=cores[c]['rinT'], nfsT=cores[c]['nfsT'],
                 sh9=cores[c]['sh9'], S=cores[c]['S'], nfT=cores[c]['nfT'],
                 rec=cores[c]['rec'])
        in_maps.append(m)

    nc = _get_nc(plan)
    res = run_bass_kernel_spmd(nc, in_maps, core_ids=list(range(N_CORES)))
    LAST_RESULT = res
    outs = [res.results[c]['out'][:NODES_PER_CORE] for c in range(N_CORES)]
    out_kap = np.concatenate(outs, axis=0).astype(np.float32)
    return ref_from_kap(out_kap)


if __name__ == "__main__":
    plan = build_plan()
    print(f"aa blocks: {len(plan.aa)}  qa blocks: {len(plan.qa)}")
    print(f"aa groups: {len(plan.aa_groups)}  qa groups: {len(plan.qa_groups)}")
    print(f"omega cols: {plan.totc}  ({plan.totc * 128 * 2 / 1e6:.1f} MB bf16)")
    print(f"emissions per node-tile: {plan.n_emi}")


# revision 33
# speedup vs baseline: 1.0029x; 1.0029x over previous
"""Trainium2 Bass kernel for nn_MACEConvolutionLayer.

Strategy (8 NeuronCores, no collectives):
  - Edges sharded by destination-node range (1250 nodes/core), sorted and
    packed into 10 windows of 128 nodes x 1024 edge slots per core. Messages
    are segment-summed into node windows via host-precomputed one-hot
    scatter matmuls on the tensor engine.
  - Per-edge bilinear (radial features x embedded source scalars) and the
    per-node equivariant tensor products use a monomial scheme computed
    directly in transposed [uv, sample] layout: replicated factor tiles
    (built by DMA through a DRAM scratch roundtrip) are multiplied
    elementwise on DVE/GpSimd, and the tensor engine contracts the monomial
    chunks against packed combined weight matrices (Clebsch-Gordan x TP
    weights with channel mixing/combination folded in). This avoids all
    PE-transposes and PSUM evacuation copies of the previous scheme.
  - Output q|msg columns are interleaved per kappa-component so each
    (block, chunk) usually emits one contiguous column run.

Feature layout on device is kappa-major: col(l, i, u) = LOFF[l] + i*32 + u.
"""
import sys, os

sys.path.insert(0, '/opt/trn_rl_repo')

import numpy as np
import ml_dtypes

MUL = 32
DIMS = (1, 3, 5)
HID = 288
N_NODES = 10000
N_EDGES = 64000
RHID = 64
SQM = float(np.sqrt(MUL))
LOFF = [0, 32, 128]
SOFF = [0, 1, 4]
PATHS_FULL = [(0,0,0),(0,1,1),(0,2,2),(1,0,1),(1,1,0),(1,1,2),(1,2,1),(2,0,2),(2,1,1),(2,2,0),(2,2,2)]
O2_UVW = [(0,1,1),(0,2,2),(1,2,1)]
O2_UVU = [(0,0,0),(1,1,0),(1,1,2),(2,2,0),(2,2,2)]

N_CORES = 8
NODES_PER_CORE = 1250
WIN = 128
N_WIN = 10
ESLOT = 1024
E_PAD = N_WIN * ESLOT   # 10240
ET_PER_WIN = ESLOT // 128  # 8
BF = ml_dtypes.bfloat16

NCOMP = 9  # number of (l, i) components
MAX_JRUN = 2   # max J-run length per product op


def comp_ord(l, i):
    return LOFF[l] // 32 + i


COMP_L = [0, 1, 1, 1, 2, 2, 2, 2, 2]  # l of each component ordinal


def cg_np():
    s2, s3, s5, s6 = map(np.sqrt, (2.0, 3.0, 5.0, 6.0))
    B = np.zeros((5, 3, 3))
    B[0, 0, 1] = B[0, 1, 0] = 1 / s2
    B[1, 1, 2] = B[1, 2, 1] = 1 / s2
    B[2] = np.diag([-1.0, -1.0, 2.0]) / s6
    B[3, 0, 2] = B[3, 2, 0] = 1 / s2
    B[4] = np.diag([1.0, -1.0, 0.0]) / s2
    C = {}
    C[(0, 0, 0)] = np.ones((1, 1, 1))
    C[(0, 1, 1)] = (np.eye(3) / s3)[None]
    C[(1, 0, 1)] = np.transpose(C[(0, 1, 1)], (1, 0, 2))
    C[(0, 2, 2)] = (np.eye(5) / s5)[None]
    C[(2, 0, 2)] = np.transpose(C[(0, 2, 2)], (1, 0, 2))
    C[(1, 1, 0)] = (np.eye(3) / s3)[:, :, None]
    C[(1, 1, 2)] = np.transpose(B, (1, 2, 0)) / s5
    C[(1, 2, 1)] = np.transpose(B, (1, 0, 2)) / s5
    C[(2, 1, 1)] = B / s5
    C[(2, 2, 0)] = (np.eye(5) / s5)[:, :, None]
    T = np.einsum('aij,bjk,cki->abc', B, B, B)
    C[(2, 2, 2)] = T / np.linalg.norm(T)
    return C


CG = cg_np()
PATH_LIST_O2 = O2_UVW + O2_UVU


def support_pairs(path_ijk):
    d = {}
    for pi, (li, lj, lk) in enumerate(path_ijk):
        C = CG[(li, lj, lk)]
        for iloc in range(DIMS[li]):
            for jloc in range(DIMS[lj]):
                if np.any(np.abs(C[iloc, jloc, :]) > 1e-12):
                    d.setdefault(((li, iloc), (lj, jloc)), []).append((pi, iloc, jloc))
    return d


def build_mono_blocks_sym(path_ijk):
    d = support_pairs(path_ijk)
    blocks = {}
    for (I, J), lst in d.items():
        key = (min(I, J), max(I, J))
        swap = I > J
        for (pi, iloc, jloc) in lst:
            blocks.setdefault(key, []).append((pi, iloc, jloc, swap))
    return [(I, J, c) for (I, J), c in sorted(blocks.items())]


def build_mono_blocks(path_ijk):
    d = support_pairs(path_ijk)
    return [(I, J, [(pi, i, j, False) for (pi, i, j) in lst]) for (I, J), lst in sorted(d.items())]


def omega_for_block(path_ijk, weights, I, J, contribs, reg):
    """[1024 (u-major,v-fast), 576] interleaved outputs:
    col(g_out, reg, w) = g_out*64 + reg*32 + w."""
    Om = np.zeros((MUL * MUL, 2 * HID))
    for (pi, iloc, jloc, swap) in contribs:
        li, lj, lk = path_ijk[pi]
        W = weights[pi]
        C = CG[(li, lj, lk)]
        for kap in range(DIMS[lk]):
            c = C[iloc, jloc, kap]
            if abs(c) < 1e-12:
                continue
            gk = comp_ord(lk, kap)
            c0 = gk * 64 + reg * 32
            Wm = W if not swap else np.transpose(W, (1, 0, 2))
            Om[:, c0:c0 + 32] += c * Wm.reshape(MUL * MUL, MUL)
    return Om


# ---------------------------------------------------------------------------
# static plan
# ---------------------------------------------------------------------------

class Plan:
    pass


def _emissions(mask):
    """mask: [1024, 576] bool. Returns per kc: list of (c0, c1) col runs
    (gaptol 0 at 32-col-slot granularity, split at 512-wide)."""
    out = []
    for kc in range(8):
        sub = mask[kc * 128:(kc + 1) * 128]
        slots = [s for s in range(18) if np.any(sub[:, s * 32:(s + 1) * 32])]
        runs = []
        for s in slots:
            if runs and s == runs[-1][1]:
                runs[-1][1] = s + 1
            else:
                runs.append([s, s + 1])
        emis = []
        for (a, b) in runs:
            while (b - a) * 32 > 512:
                emis.append((a * 32, a * 32 + 512))
                a += 16
            emis.append((a * 32, b * 32))
        out.append(emis)
    return out


def build_plan():
    p = Plan()
    aa_blocks = build_mono_blocks_sym(PATHS_FULL + PATH_LIST_O2)
    qa_blocks = build_mono_blocks(PATHS_FULL)
    n3a = len(PATHS_FULL)
    ones_a = [np.ones((MUL, MUL, MUL)) for _ in PATHS_FULL]
    ones_o2 = [np.ones((MUL, MUL, MUL)) for _ in PATH_LIST_O2]

    p.aa = []
    for (I, J, contribs) in aa_blocks:
        cq = [(pi, i, j, s) for (pi, i, j, s) in contribs if pi < n3a]
        cm = [(pi - n3a, i, j, s) for (pi, i, j, s) in contribs if pi >= n3a]
        mask = np.zeros((1024, 576), bool)
        if cq:
            mask |= omega_for_block(PATHS_FULL, ones_a, I, J, cq, 0) != 0
        if cm:
            mask |= omega_for_block(PATH_LIST_O2, ones_o2, I, J, cm, 1) != 0
        p.aa.append((I, J, cq, cm, _emissions(mask)))
    p.qa = []
    for (I, J, contribs) in qa_blocks:
        mask = omega_for_block(PATHS_FULL, ones_a, I, J, contribs, 1) != 0
        p.qa.append((I, J, contribs, _emissions(mask)))

    # omega column offsets
    off = 0
    p.aa_emi = []
    for (I, J, cq, cm, em) in p.aa:
        bk = []
        for kc in range(8):
            lst = []
            for (c0, c1) in em[kc]:
                lst.append((c0, c1, off))
                off += c1 - c0
            bk.append(lst)
        p.aa_emi.append(bk)
    p.qa_emi = []
    for (I, J, contribs, em) in p.qa:
        bk = []
        for kc in range(8):
            lst = []
            for (c0, c1) in em[kc]:
                lst.append((c0, c1, off))
                off += c1 - c0
            bk.append(lst)
        p.qa_emi.append(bk)
    p.totc = off
    p.n_emi = sum(len(l) for bk in p.aa_emi + p.qa_emi for l in bk)

    # J-run groups for product ops: consecutive blocks with same I and
    # consecutive J ordinals, capped at MAX_JRUN
    def groups(blocks):
        gs = []
        for bi, blk in enumerate(blocks):
            I, J = blk[0], blk[1]
            gI = comp_ord(*I); gJ = comp_ord(*J)
            if (gs and gs[-1][0] == gI and gs[-1][1] + gs[-1][2] == gJ
                    and gs[-1][2] < MAX_JRUN):
                gs[-1][2] += 1
            else:
                gs.append([gI, gJ, 1, bi])
        return [(gI, gJ, n, b0) for (gI, gJ, n, b0) in gs]

    p.aa_groups = groups(p.aa)
    p.qa_groups = groups(p.qa)
    return p


def pack_omega(plan, Wfold):
    W3a = Wfold['o3a_w']; Wo2 = Wfold['o2_w']; W3b = Wfold['o3b_w']
    om = np.zeros((128, plan.totc), np.float32)
    for bi, (I, J, cq, cm, em) in enumerate(plan.aa):
        Om = np.zeros((MUL * MUL, 2 * HID))
        if cq:
            Om += omega_for_block(PATHS_FULL, W3a, I, J, cq, 0)
        if cm:
            Om += omega_for_block(PATH_LIST_O2, Wo2, I, J, cm, 1)
        for kc in range(8):
            for (c0, c1, off) in plan.aa_emi[bi][kc]:
                om[:, off:off + (c1 - c0)] = Om[kc * 128:(kc + 1) * 128, c0:c1]
    for bi, (I, J, contribs, em) in enumerate(plan.qa):
        Om = omega_for_block(PATHS_FULL, W3b, I, J, contribs, 1)
        for kc in range(8):
            for (c0, c1, off) in plan.qa_emi[bi][kc]:
                om[:, off:off + (c1 - c0)] = Om[kc * 128:(kc + 1) * 128, c0:c1]
    return om.astype(BF)


def fold_weights(inp):
    f8 = np.float64
    mix_w = inp['mix_w'].astype(f8); comb_w = inp['comb_w'].astype(f8)
    M = np.einsum('olux,olxw->oluw', mix_w, comb_w) / MUL
    W1eff = np.einsum('lux,lxw->luw', inp['lin_o1'].astype(f8), M[0]) / SQM
    o2_w = []
    for pp, (i, j, k) in enumerate(O2_UVW):
        o2_w.append(np.einsum('uvx,xw->uvw', inp['o2_uvw'][pp].astype(f8) / MUL, M[1][k]))
    for pp, (i, j, k) in enumerate(O2_UVU):
        o2_w.append(np.einsum('uv,uw->uvw', inp['o2_uvu'][pp].astype(f8), M[1][k]) / SQM)
    o3a_w = [inp['o3a_uvw'][pp].astype(f8) / MUL for pp in range(len(PATHS_FULL))]
    o3b_w = [np.einsum('uvx,xw->uvw', inp['o3b_uvw'][pp].astype(f8) / MUL, M[2][k])
             for pp, (i, j, k) in enumerate(PATHS_FULL)]
    aw = inp['a_w'].astype(f8).reshape(RHID, 3, MUL, MUL)
    ab = inp['a_b'].astype(f8).reshape(3, MUL, MUL)
    scale = np.array([1.0 / np.sqrt(d) for d in DIMS]) / SQM
    aw = aw * scale[None, :, None, None]
    ab = ab * scale[:, None, None]
    A2 = np.transpose(aw, (0, 2, 1, 3)).reshape(RHID * MUL, 3 * MUL)
    B2 = np.transpose(ab, (1, 0, 2)).reshape(MUL, 3 * MUL)
    # omc1: [32, 3*32]: per-l 32x32 order-1 linear (same for all i of that l)
    omc1 = np.zeros((32, 96))
    for l in range(3):
        omc1[:, l * 32:(l + 1) * 32] = W1eff[l]
    return dict(
        o3a_w=o3a_w, o2_w=o2_w, o3b_w=o3b_w,
        omc1=omc1, omself=inp['self_w'].astype(f8) / SQM,
        emb=inp['emb_w'].astype(f8) / SQM,
        A2=A2, B2=B2,
        r_w1=inp['r_w1'].astype(np.float32), r_b1=inp['r_b1'].astype(np.float32),
        r_w2=inp['r_w2'].astype(np.float32), r_b2=inp['r_b2'].astype(np.float32),
        r_w3=inp['r_w3'].astype(np.float32), r_b3=inp['r_b3'].astype(np.float32),
    )


def pack_edges(inp):
    src = np.asarray(inp['edge_index'][0]).astype(np.int64)
    dst = np.asarray(inp['edge_index'][1]).astype(np.int64)
    sh = np.asarray(inp['edge_sh'], dtype=np.float32)
    rad = np.asarray(inp['edge_radial_embedding'], dtype=np.float32)
    attr = np.asarray(inp['edge_attr'], dtype=np.float32)
    nf = np.asarray(inp['node_features'], dtype=np.float32)
    cnt = np.bincount(dst, minlength=N_NODES).astype(np.float32)
    rec_all = 1.0 / np.maximum(cnt, 1.0)
    order = np.argsort(dst, kind='stable')
    dst_s = dst[order]
    cores = []
    for c in range(N_CORES):
        lo = c * NODES_PER_CORE
        rinT = np.zeros((24, E_PAD), np.float32)
        nfsT = np.zeros((MUL, E_PAD), np.float32)
        sh9 = np.zeros((E_PAD, 9), np.float32)
        S = np.zeros((E_PAD, 128), BF)
        for w in range(N_WIN):
            nlo = lo + w * WIN
            nhi = min(lo + (w + 1) * WIN, lo + NODES_PER_CORE)
            a = np.searchsorted(dst_s, nlo); b = np.searchsorted(dst_s, nhi)
            idx = order[a:b]
            n = b - a
            assert n <= ESLOT, f"window overflow {n}"
            s = w * ESLOT
            rinT[:8, s:s + n] = rad[idx].T
            rinT[8:, s:s + n] = attr[idx].T
            nfsT[:, s:s + n] = nf[src[idx]].T
            sh9[s:s + n, :] = sh[idx]
            S[s + np.arange(n), (dst[idx] - nlo)] = BF(1.0)
        nfT = np.zeros((MUL, N_WIN * WIN), BF)
        nfT[:, :NODES_PER_CORE] = nf[lo:lo + NODES_PER_CORE].T.astype(BF)
        rec = np.ones((N_WIN * WIN, 1), np.float32)
        rec[:NODES_PER_CORE, 0] = rec_all[lo:lo + NODES_PER_CORE]
        cores.append(dict(rinT=rinT, nfsT=nfsT, sh9=sh9, S=S, nfT=nfT, rec=rec))
    return cores


def ref_from_kap(x_kap):
    out = np.empty_like(x_kap)
    for l, d in enumerate(DIMS):
        blk = x_kap[:, LOFF[l]:LOFF[l] + 32 * d].reshape(-1, d, 32)
        out[:, LOFF[l]:LOFF[l] + 32 * d] = np.transpose(blk, (0, 2, 1)).reshape(-1, 32 * d)
    return out


# ---------------------------------------------------------------------------
# device kernel
# ---------------------------------------------------------------------------

_NC_CACHE = {}
LAST_RESULT = None

# fraction of product work sent to gpsimd (tuned from profiles)
GP_ELEM_NS = 99.0e-3   # us per free-elem (effectively disable gpsimd)
VE_ELEM_NS = 0.52e-3
GP_OP_OH = 0.25
VE_OP_OH = 0.08


def build_nc(plan):
    import concourse.bass as bass
    import concourse.bacc as bacc
    import concourse.mybir as mybir
    import concourse.tile as tile

    f32 = mybir.dt.float32
    bf16 = mybir.dt.bfloat16
    AL = mybir.AluOpType
    AF = mybir.ActivationFunctionType

    nc = bacc.Bacc(None)
    P = 128

    # ---- dram parameters
    rinT_d = nc.declare_dram_parameter("rinT", [24, E_PAD], f32, isOutput=False)
    nfsT_d = nc.declare_dram_parameter("nfsT", [32, E_PAD], f32, isOutput=False)
    sh9_d = nc.declare_dram_parameter("sh9", [E_PAD, 9], f32, isOutput=False)
    S_d = nc.declare_dram_parameter("S", [E_PAD, 128], bf16, isOutput=False)
    nfT_d = nc.declare_dram_parameter("nfT", [32, N_WIN * WIN], bf16, isOutput=False)
    rec_d = nc.declare_dram_parameter("rec", [N_WIN * WIN, 1], f32, isOutput=False)
    omega_d = nc.declare_dram_parameter("omega", [P, plan.totc], bf16, isOutput=False)
    a2_d = nc.declare_dram_parameter("a2", [P, 16 * 96], bf16, isOutput=False)
    b2_d = nc.declare_dram_parameter("b2", [32, 96], bf16, isOutput=False)
    omc1_d = nc.declare_dram_parameter("omc1", [32, 96], bf16, isOutput=False)
    omself_d = nc.declare_dram_parameter("omself", [32, 32], bf16, isOutput=False)
    rw1_d = nc.declare_dram_parameter("rw1", [24, 64], f32, isOutput=False)
    rw2_d = nc.declare_dram_parameter("rw2", [64, 64], f32, isOutput=False)
    rw3_d = nc.declare_dram_parameter("rw3", [64, 64], f32, isOutput=False)
    rb1_d = nc.declare_dram_parameter("rb1", [64, 1], f32, isOutput=False)
    rb2_d = nc.declare_dram_parameter("rb2", [64, 1], f32, isOutput=False)
    emb_d = nc.declare_dram_parameter("emb", [32, 32], f32, isOutput=False)
    identb_d = nc.declare_dram_parameter("identb", [P, P], bf16, isOutput=False)
    selfull_d = nc.declare_dram_parameter("selfull", [P, 1024], bf16, isOutput=False)
    selr_d = nc.declare_dram_parameter("selr", [64, 2048], bf16, isOutput=False)
    repfull_d = nc.declare_dram_parameter("repfull", [P, P], bf16, isOutput=False)
    zer_d = nc.declare_dram_parameter("zer", [1, P], bf16, isOutput=False)
    zer2_d = nc.declare_dram_parameter("zer2", [1, 2 * HID], bf16, isOutput=False)
    out_d = nc.declare_dram_parameter("out", [N_WIN * WIN, HID], f32, isOutput=True)

    # engine schedule for product ops: greedy balance vector vs gpsimd
    def make_sched():
        ops = []
        for gi, (gI, gJ, nJ, b0) in enumerate(plan.aa_groups):
            ops.append(('aa', gi, nJ * 1024))
        for gi, (gI, gJ, nJ, b0) in enumerate(plan.qa_groups):
            ops.append(('qa', gi, nJ * 1024))
        for q in range(4):
            ops.append(('edge', q, 4096))
        v_t, g_t = 1.5, 0.0   # vector pre-loaded with msgs/evac budget
        sched = {}
        for (kind, idx, wdt) in ops:
            vc = wdt * VE_ELEM_NS + VE_OP_OH
            gc = wdt * GP_ELEM_NS + GP_OP_OH
            if g_t + gc < v_t + vc:
                sched[(kind, idx)] = 'gpsimd'; g_t += gc
            else:
                sched[(kind, idx)] = 'vector'; v_t += vc
        return sched

    sched = make_sched()

    from contextlib import ExitStack
    with tile.TileContext(nc) as tc, ExitStack() as es:
        cst = es.enter_context(tc.tile_pool(name="cst", bufs=1))
        sb2 = es.enter_context(tc.tile_pool(name="sb2", bufs=2))
        sb3 = es.enter_context(tc.tile_pool(name="sb3", bufs=2))
        uu_pool = es.enter_context(tc.tile_pool(name="uu", bufs=1))
        pt_pool = es.enter_context(tc.tile_pool(name="pt", bufs=8))
        ed_pool = es.enter_context(tc.tile_pool(name="ed", bufs=1))
        sb1 = es.enter_context(tc.tile_pool(name="sb1", bufs=1))
        ps_wps = es.enter_context(tc.tile_pool(name="pswps", bufs=1, space="PSUM"))
        ps_uub = es.enter_context(tc.tile_pool(name="psuub", bufs=2, space="PSUM"))
        ps_qm = es.enter_context(tc.tile_pool(name="psqm", bufs=1, space="PSUM"))
        ps_tp = es.enter_context(tc.tile_pool(name="pstp", bufs=1, space="PSUM"))
        ps_mlp = es.enter_context(tc.tile_pool(name="psmlp", bufs=1, space="PSUM"))
        ps_mx = es.enter_context(tc.tile_pool(name="psmx", bufs=1, space="PSUM"))

        # ---- constants
        omega = cst.tile([P, plan.totc], bf16)
        nc.sync.dma_start(out=omega[:], in_=omega_d[:])
        a2 = cst.tile([P, 16 * 96], bf16)
        nc.sync.dma_start(out=a2[:], in_=a2_d[:])
        b2 = cst.tile([32, 96], bf16); nc.sync.dma_start(out=b2[:], in_=b2_d[:])
        omc1 = cst.tile([32, 96], bf16); nc.sync.dma_start(out=omc1[:], in_=omc1_d[:])
        omself = cst.tile([32, 32], bf16); nc.sync.dma_start(out=omself[:], in_=omself_d[:])
        rw1 = cst.tile([24, 64], f32); nc.sync.dma_start(out=rw1[:], in_=rw1_d[:])
        rw2 = cst.tile([64, 64], f32); nc.sync.dma_start(out=rw2[:], in_=rw2_d[:])
        rw3 = cst.tile([64, 64], f32); nc.sync.dma_start(out=rw3[:], in_=rw3_d[:])
        rb1 = cst.tile([64, 1], f32); nc.sync.dma_start(out=rb1[:], in_=rb1_d[:])
        rb2 = cst.tile([64, 1], f32); nc.sync.dma_start(out=rb2[:], in_=rb2_d[:])
        emb = cst.tile([32, 32], f32); nc.sync.dma_start(out=emb[:], in_=emb_d[:])
        identb = cst.tile([P, P], bf16); nc.sync.dma_start(out=identb[:], in_=identb_d[:])
        selfull = cst.tile([P, 1024], bf16); nc.sync.dma_start(out=selfull[:], in_=selfull_d[:])
        selr = cst.tile([64, 2048], bf16); nc.sync.dma_start(out=selr[:], in_=selr_d[:])
        repfull = cst.tile([P, P], bf16); nc.sync.dma_start(out=repfull[:], in_=repfull_d[:])
        zer = cst.tile([1, P], bf16); nc.sync.dma_start(out=zer[:], in_=zer_d[:])
        zer2 = cst.tile([1, 2 * HID], bf16); nc.sync.dma_start(out=zer2[:], in_=zer2_d[:])
        nfT = cst.tile([32, N_WIN * WIN], bf16)
        nc.sync.dma_start(out=nfT[:], in_=nfT_d[:])

        def transpose3(x_bf, tag):
            """x_bf [128, 288] bf16 -> aT sbuf [128, 384] (chunk-major)."""
            tp = ps_tp.tile([P, 384], bf16, space="PSUM", tag="tp")
            nc.tensor.transpose(out=tp[:, 0:P], in_=x_bf[:, 0:P], identity=identb[:])
            nc.tensor.transpose(out=tp[:, P:2 * P], in_=x_bf[:, P:2 * P], identity=identb[:])
            nc.tensor.transpose(out=tp[0:32, 2 * P:3 * P], in_=x_bf[:, 2 * P:HID], identity=identb[:])
            xt = sb2.tile([P, 384], bf16, tag=tag + "sb")
            nc.scalar.copy(out=xt[:, 0:2 * P], in_=tp[:, 0:2 * P])
            nc.scalar.copy(out=xt[0:32, 2 * P:3 * P], in_=tp[0:32, 2 * P:3 * P])
            return xt

        NCC = [3, 2, 2, 2]   # comps per partition-row-group b: g = 4*cc + b <= 8

        def build_uu(aT, uu_tile, ev):
            """uu[32*u4+v, (g,kc,n)] = aT-val[f=32g+4kc+u4, node n] via SEL matmuls."""
            for b in range(4):
                ncc = NCC[b]
                for kc in range(8):
                    up = ps_uub.tile([P, 512], f32, space="PSUM", tag="uub")
                    nc.tensor.matmul(out=up[:, :ncc * P],
                                     lhsT=selfull[32 * b:32 * (b + 1), kc * P:(kc + 1) * P],
                                     rhs=aT[32 * b:32 * (b + 1), :ncc * P],
                                     start=True, stop=True, tile_position=(32 * b, 0))
                    dst = uu_tile[:].rearrange("p (g k n) -> p g k n", k=8, n=P)[:, b::4, kc, :]
                    src_ = up[:, :ncc * P].rearrange("p (c n) -> p c n", n=P)
                    nc.scalar.copy(out=dst, in_=src_)

        def build_v8(aT, v8_tile, ev):
            """v8[32*b+v, (g,n)] = aT-val[f=32g+v, node n] (mod-32 replication)."""
            for b in range(4):
                ncc = NCC[b]
                up = ps_uub.tile([P, 512], f32, space="PSUM", tag="uub")
                nc.tensor.matmul(out=up[:, :ncc * P],
                                 lhsT=repfull[32 * b:32 * (b + 1), :],
                                 rhs=aT[32 * b:32 * (b + 1), :ncc * P],
                                 start=True, stop=True, tile_position=(32 * b, 0))
                dst = v8_tile[:].rearrange("p (g n) -> p g n", n=P)[:, b::4, :]
                src_ = up[:, :ncc * P].rearrange("p (c n) -> p c n", n=P)
                nc.scalar.copy(out=dst, in_=src_)

        def emit_product_group(gi, groups, blocks, emi, uu, v8, qm, kind):
            (gI, gJ, nJ, b0) = groups[gi]
            wdt = nJ * 1024
            PT = pt_pool.tile([P, MAX_JRUN * 1024], bf16, tag="PT")
            eng = nc.gpsimd if sched[(kind, gi)] == 'gpsimd' else nc.vector
            eng.tensor_tensor(
                out=PT[:, :wdt].rearrange("p (j k n) -> p j k n", k=8, n=P),
                in0=uu[:, gI * 1024:(gI + 1) * 1024]
                    .rearrange("p (k n) -> p k n", n=P)[:, None, :, :]
                    .broadcast_to([P, nJ, 8, P]),
                in1=v8[:, gJ * P:(gJ + nJ) * P]
                    .rearrange("p (j n) -> p j n", n=P)[:, :, None, :]
                    .broadcast_to([P, nJ, 8, P]),
                op=AL.mult)
            for jl in range(nJ):
                bi = b0 + jl
                for kc in range(8):
                    for (c0, c1, off) in emi[bi][kc]:
                        nc.tensor.matmul(out=qm[:, c0:c1],
                                         lhsT=PT[:, jl * 1024 + kc * P: jl * 1024 + (kc + 1) * P],
                                         rhs=omega[:, off:off + (c1 - c0)],
                                         start=False, stop=False,
                                         skip_group_check=True)
        # per-window state for the software pipeline
        st = {}

        def emit_mlp_half(w, h):
            e0 = w * ESLOT
            if h == 0:
                rfT_t = sb1.tile([64, ESLOT], bf16, tag="rfT")
                hT_t = sb1.tile([32, ESLOT], bf16, tag="hT")
                vh_t = sb1.tile([P, ESLOT], bf16, tag="vh")
                wps_t = ps_wps.tile([P, HID], f32, space="PSUM", tag="wps")
                st[w] = dict(rfT=rfT_t, hT=hT_t, vh=vh_t, wps=wps_t, ev=[0])
            S = st[w]
            s = e0 + h * 512
            rin_h = sb2.tile([24, 512], f32, tag="rin")
            nc.sync.dma_start(out=rin_h[:], in_=rinT_d[:, s:s + 512])
            nfs_h = sb2.tile([32, 512], f32, tag="nfs")
            nc.sync.dma_start(out=nfs_h[:], in_=nfsT_d[:, s:s + 512])
            l1p = ps_mlp.tile([64, 512], f32, space="PSUM", tag="mlp")
            nc.tensor.matmul(out=l1p[:], lhsT=rw1[:], rhs=rin_h[:], start=True, stop=True)
            f1 = sb2.tile([64, 512], f32, tag="f")
            nc.scalar.activation(out=f1[:], in_=l1p[:], func=AF.Silu, bias=rb1[:], scale=1.0)
            l2p = ps_mlp.tile([64, 512], f32, space="PSUM", tag="mlp")
            nc.tensor.matmul(out=l2p[:], lhsT=rw2[:], rhs=f1[:], start=True, stop=True)
            f2 = sb2.tile([64, 512], f32, tag="f")
            nc.scalar.activation(out=f2[:], in_=l2p[:], func=AF.Silu, bias=rb2[:], scale=1.0)
            rfp = ps_mlp.tile([64, 512], f32, space="PSUM", tag="mlp")
            nc.tensor.matmul(out=rfp[:], lhsT=rw3[:], rhs=f2[:], start=True, stop=True)
            nc.scalar.copy(out=S['rfT'][:, h * 512:(h + 1) * 512], in_=rfp[:])
            hp = ps_mlp.tile([32, 512], f32, space="PSUM", tag="mlp")
            nc.tensor.matmul(out=hp[:], lhsT=emb[:], rhs=nfs_h[:], start=True, stop=True)
            nc.scalar.copy(out=S['hT'][:, h * 512:(h + 1) * 512], in_=hp[:])
            for b in range(4):
                nc.scalar.dma_start(out=S['vh'][32 * b:32 * (b + 1), h * 512:(h + 1) * 512],
                                    in_=S['hT'][:, h * 512:(h + 1) * 512])

        def emit_edge_quarter(w, q):
            e0 = w * ESLOT
            S = st[w]
            rfT, hT, vh, wps = S['rfT'], S['hT'], S['vh'], S['wps']
            uurf = ed_pool.tile([P, 4096], bf16, tag="uurf")
            for cp in range(8):
                up = ps_uub.tile([P, 512], f32, space="PSUM", tag="uub")
                for ci in range(2):
                    c = cp * 2 + ci
                    nc.tensor.matmul(out=up[:, ci * 256:(ci + 1) * 256],
                                     lhsT=selr[:, c * P:(c + 1) * P],
                                     rhs=rfT[:, q * 256:(q + 1) * 256],
                                     start=True, stop=True)
                nc.scalar.copy(out=uurf[:, cp * 512:(cp + 1) * 512], in_=up[:])
            mT = ed_pool.tile([P, 4096], bf16, tag="mT")
            eng = nc.gpsimd if sched[('edge', q)] == 'gpsimd' else nc.vector
            eng.tensor_tensor(
                out=mT[:].rearrange("p (c e) -> p c e", e=256),
                in0=uurf[:].rearrange("p (c e) -> p c e", e=256),
                in1=vh[:, q * 256:(q + 1) * 256][:, None, :].broadcast_to([P, 16, 256]),
                op=AL.mult)
            for tt in range(2):
                t = q * 2 + tt
                et = e0 + t * P
                mxp = ps_mx.tile([P, 96], f32, space="PSUM", tag="mx")
                for c in range(16):
                    nc.tensor.matmul(out=mxp[:], lhsT=mT[:, c * 256 + tt * P:c * 256 + (tt + 1) * P],
                                     rhs=a2[:, c * 96:(c + 1) * 96],
                                     start=(c == 0), stop=False)
                nc.tensor.matmul(out=mxp[:], lhsT=hT[:, t * P:(t + 1) * P], rhs=b2[:],
                                 start=False, stop=True)
                sh_t = sb3.tile([P, 9], f32, tag="sht")
                nc.sync.dma_start(out=sh_t[:], in_=sh9_d[et:et + P, :])
                msgs = sb2.tile([P, HID], bf16, tag="msgs")
                for l, d in enumerate(DIMS):
                    nc.vector.tensor_tensor(
                        out=msgs[:, LOFF[l]:LOFF[l] + 32 * d].rearrange("p (i u) -> p i u", u=32),
                        in0=sh_t[:, SOFF[l]:SOFF[l] + d][:, :, None].broadcast_to([P, d, 32]),
                        in1=mxp[:, l * 32:(l + 1) * 32][:, None, :].broadcast_to([P, d, 32]),
                        op=AL.mult)
                S_t = sb3.tile([P, P], bf16, tag="St")
                nc.sync.dma_start(out=S_t[:], in_=S_d[et:et + P, :])
                nc.tensor.matmul(out=wps[:], lhsT=S_t[:], rhs=msgs[:],
                                 start=(t == 0), stop=(t == ET_PER_WIN - 1))

        def emit_node_prefix(w):
            S = st[w]
            rec_t = sb2.tile([P, 1], f32, tag="rec")
            nc.sync.dma_start(out=rec_t[:], in_=rec_d[w * P:(w + 1) * P, :])
            a_bf = sb2.tile([P, HID], bf16, tag="abf")
            nc.vector.tensor_scalar_mul(out=a_bf[:], in0=S['wps'][:], scalar1=rec_t[:])
            aT = transpose3(a_bf, "at")
            uu = uu_pool.tile([P, NCOMP * 1024], bf16, tag="uu")
            build_uu(aT, uu, S['ev'])
            v8 = sb2.tile([P, NCOMP * P], bf16, tag="v8")
            build_v8(aT, v8, S['ev'])
            qm = ps_qm.tile([P, 2 * HID], f32, space="PSUM", tag="qm")
            nc.tensor.matmul(out=qm[:, 0:512], lhsT=zer[:], rhs=zer2[:, 0:512], start=True, stop=False, skip_group_check=True)
            nc.tensor.matmul(out=qm[:, 512:576], lhsT=zer[:], rhs=zer2[:, 512:576], start=True, stop=False, skip_group_check=True)
            S.update(uu=uu, v8=v8, qm=qm)

        def emit_node_qmid(w):
            S = st[w]
            q_bf = sb2.tile([P, HID], bf16, tag="qbf")
            nc.vector.tensor_copy(
                out=q_bf[:].rearrange("p (g c) -> p g c", c=32),
                in_=S['qm'][:].rearrange("p (g t c) -> p g t c", t=2, c=32)[:, :, 0, :])
            qT = transpose3(q_bf, "qt")
            uuq = uu_pool.tile([P, NCOMP * 1024], bf16, tag="uu")
            build_uu(qT, uuq, S['ev'])
            S.update(uuq=uuq)

        def emit_node_suffix(w):
            S = st[w]
            qm, v8 = S['qm'], S['v8']
            for g in range(NCOMP):
                l = COMP_L[g]
                nc.tensor.matmul(out=qm[:, g * 64 + 32:g * 64 + 64],
                                 lhsT=v8[0:32, g * P:(g + 1) * P],
                                 rhs=omc1[:, l * 32:(l + 1) * 32],
                                 start=False, stop=False, skip_group_check=True)
            nc.tensor.matmul(out=qm[:, 32:64], lhsT=nfT[:, w * P:(w + 1) * P],
                             rhs=omself[:], start=False, stop=True,
                             skip_group_check=True)
            out_sb = sb1.tile([P, HID], f32, tag="outsb")
            nc.scalar.copy(
                out=out_sb[:].rearrange("p (g c) -> p g c", c=32),
                in_=qm[:].rearrange("p (g t c) -> p g t c", t=2, c=32)[:, :, 1, :])
            nc.sync.dma_start(out=out_d[w * P:(w + 1) * P, :], in_=out_sb[:])
            del st[w]

        # ---------------- software-pipelined main loop ----------------
        # edge phase of window w+1 is interleaved into node phase of window w
        # so the PE queue always has ready work (keeps HAM warm).
        
        def node_events(w):
            ev = []
            ev.append(lambda w=w: emit_node_prefix(w))
            aa_order = sorted(range(len(plan.aa_groups)),
                              key=lambda gi: (plan.aa_groups[gi][0] % 4, plan.aa_groups[gi][0] // 4))
            for gi in aa_order:
                ev.append(lambda w=w, gi=gi: emit_product_group(gi, plan.aa_groups, plan.aa, plan.aa_emi, st[w]['uu'], st[w]['v8'], st[w]['qm'], 'aa'))
            ev.append(lambda w=w: emit_node_qmid(w))
            qa_order = sorted(range(len(plan.qa_groups)),
                              key=lambda gi: (plan.qa_groups[gi][0] % 4, plan.qa_groups[gi][0] // 4))
            for gi in qa_order:
                ev.append(lambda w=w, gi=gi: emit_product_group(gi, plan.qa_groups, plan.qa, plan.qa_emi, st[w]['uuq'], st[w]['v8'], st[w]['qm'], 'qa'))
            ev.append(lambda w=w: emit_node_suffix(w))
            return ev

        def edge_events(w):
            ev = [lambda w=w: emit_mlp_half(w, 0),
                  lambda w=w: emit_edge_quarter(w, 0),
                  lambda w=w: emit_edge_quarter(w, 1),
                  lambda w=w: emit_mlp_half(w, 1),
                  lambda w=w: emit_edge_quarter(w, 2),
                  lambda w=w: emit_edge_quarter(w, 3)]
            return ev

        # prologue: edge phase of window 0 runs alone
        for f in edge_events(0):
            f()
        for w in range(N_WIN):
            for f in node_events(w):
                f()
            if w + 1 < N_WIN:
                for f in edge_events(w + 1):
                    f()

    nc.finalize()
    return nc


def _get_nc(plan):
    if 'nc' not in _NC_CACHE:
        _NC_CACHE['nc'] = build_nc(plan)
    return _NC_CACHE['nc']


def kernel(**inputs):
    global LAST_RESULT
    from concourse.bass_utils import run_bass_kernel_spmd

    inp = {k: np.asarray(v) for k, v in inputs.items()}
    plan = build_plan()
    W = fold_weights(inp)
    om = pack_omega(plan, W)

    A2 = W['A2'].astype(np.float32)
    a2p = np.zeros((128, 16 * 96), np.float32)
    for c in range(16):
        a2p[:, c * 96:(c + 1) * 96] = A2[c * 128:(c + 1) * 128, :]
    # fold r_b3 into B2 (rf = f2 @ rw3; +b3 contribution is linear in h)
    B2 = W['B2'].astype(np.float64).copy()
    b3 = inp['r_b3'].astype(np.float64)
    for u in range(32):
        B2[u, :] += b3 @ A2[np.arange(RHID) * 32 + u, :].astype(np.float64)

    identb = np.eye(128, dtype=np.float32).astype(BF)
    self = None
    selfull = np.zeros((128, 1024), np.float32)
    for p in range(128):
        for kc in range(8):
            u4 = p % 32 - 4 * kc
            if 0 <= u4 < 4:
                selfull[p, kc * 128 + u4 * 32:kc * 128 + (u4 + 1) * 32] = 1.0
    selr = np.zeros((64, 2048), np.float32)
    for q in range(64):
        c, r4 = divmod(q, 4)
        selr[q, c * 128 + r4 * 32:c * 128 + (r4 + 1) * 32] = 1.0
    repfull = np.zeros((128, 128), np.float32)
    for p in range(128):
        for i in range(128):
            if i % 32 == p % 32:
                repfull[p, i] = 1.0

    shared = dict(
        omega=om,
        a2=a2p.astype(BF), b2=B2.astype(np.float32).astype(BF),
        omc1=W['omc1'].astype(np.float32).astype(BF),
        omself=W['omself'].astype(np.float32).astype(BF),
        rw1=W['r_w1'], rw2=W['r_w2'], rw3=W['r_w3'],
        rb1=W['r_b1'].reshape(64, 1), rb2=W['r_b2'].reshape(64, 1),
        emb=W['emb'].astype(np.float32),
        identb=identb,
        selfull=selfull.astype(BF), selr=selr.astype(BF), repfull=repfull.astype(BF),
        zer=np.zeros((1, 128), BF), zer2=np.zeros((1, 2 * HID), BF),
    )
    cores = pack_edges(inp)
    in_maps = []
    for c in range(N_CORES):
        m = dict(shared)
        m.update(rinT=cores[c]['rinT'], nfsT=cores[c]['nfsT'],
                 sh9=cores[c]['sh9'], S=cores[c]['S'], nfT=cores[c]['nfT'],
                 rec=cores[c]['rec'])
        in_maps.append(m)

    nc = _get_nc(plan)
    res = run_bass_kernel_spmd(nc, in_maps, core_ids=list(range(N_CORES)))
    LAST_RESULT = res
    outs = [res.results[c]['out'][:NODES_PER_CORE] for c in range(N_CORES)]
    out_kap = np.concatenate(outs, axis=0).astype(np.float32)
    return ref_from_kap(out_kap)


if __name__ == "__main__":
    plan = build_plan()
    print(f"aa blocks: {len(plan.aa)}  qa blocks: {len(plan.qa)}")
    print(f"aa groups: {len(plan.aa_groups)}  qa groups: {len(plan.qa_groups)}")
    print(f"omega cols: {plan.totc}  ({plan.totc * 128 * 2 / 1e6:.1f} MB bf16)")
    print(f"emissions per node-tile: {plan.n_emi}")


# revision 35
# speedup vs baseline: 1.0056x; 1.0027x over previous
"""Trainium2 Bass kernel for nn_MACEConvolutionLayer.

Strategy (8 NeuronCores, no collectives):
  - Edges sharded by destination-node range (1250 nodes/core), sorted and
    packed into 10 windows of 128 nodes x 1024 edge slots per core. Messages
    are segment-summed into node windows via host-precomputed one-hot
    scatter matmuls on the tensor engine.
  - Per-edge bilinear (radial features x embedded source scalars) and the
    per-node equivariant tensor products use a monomial scheme computed
    directly in transposed [uv, sample] layout: replicated factor tiles
    (built by DMA through a DRAM scratch roundtrip) are multiplied
    elementwise on DVE/GpSimd, and the tensor engine contracts the monomial
    chunks against packed combined weight matrices (Clebsch-Gordan x TP
    weights with channel mixing/combination folded in). This avoids all
    PE-transposes and PSUM evacuation copies of the previous scheme.
  - Output q|msg columns are interleaved per kappa-component so each
    (block, chunk) usually emits one contiguous column run.

Feature layout on device is kappa-major: col(l, i, u) = LOFF[l] + i*32 + u.
"""
import sys, os

sys.path.insert(0, '/opt/trn_rl_repo')

import numpy as np
import ml_dtypes

MUL = 32
DIMS = (1, 3, 5)
HID = 288
N_NODES = 10000
N_EDGES = 64000
RHID = 64
SQM = float(np.sqrt(MUL))
LOFF = [0, 32, 128]
SOFF = [0, 1, 4]
PATHS_FULL = [(0,0,0),(0,1,1),(0,2,2),(1,0,1),(1,1,0),(1,1,2),(1,2,1),(2,0,2),(2,1,1),(2,2,0),(2,2,2)]
O2_UVW = [(0,1,1),(0,2,2),(1,2,1)]
O2_UVU = [(0,0,0),(1,1,0),(1,1,2),(2,2,0),(2,2,2)]

N_CORES = 8
NODES_PER_CORE = 1250
WIN = 128
N_WIN = 10
ESLOT = 1024
E_PAD = N_WIN * ESLOT   # 10240
ET_PER_WIN = ESLOT // 128  # 8
BF = ml_dtypes.bfloat16

NCOMP = 9  # number of (l, i) components
MAX_JRUN = 2   # max J-run length per product op


def comp_ord(l, i):
    return LOFF[l] // 32 + i


COMP_L = [0, 1, 1, 1, 2, 2, 2, 2, 2]  # l of each component ordinal


def cg_np():
    s2, s3, s5, s6 = map(np.sqrt, (2.0, 3.0, 5.0, 6.0))
    B = np.zeros((5, 3, 3))
    B[0, 0, 1] = B[0, 1, 0] = 1 / s2
    B[1, 1, 2] = B[1, 2, 1] = 1 / s2
    B[2] = np.diag([-1.0, -1.0, 2.0]) / s6
    B[3, 0, 2] = B[3, 2, 0] = 1 / s2
    B[4] = np.diag([1.0, -1.0, 0.0]) / s2
    C = {}
    C[(0, 0, 0)] = np.ones((1, 1, 1))
    C[(0, 1, 1)] = (np.eye(3) / s3)[None]
    C[(1, 0, 1)] = np.transpose(C[(0, 1, 1)], (1, 0, 2))
    C[(0, 2, 2)] = (np.eye(5) / s5)[None]
    C[(2, 0, 2)] = np.transpose(C[(0, 2, 2)], (1, 0, 2))
    C[(1, 1, 0)] = (np.eye(3) / s3)[:, :, None]
    C[(1, 1, 2)] = np.transpose(B, (1, 2, 0)) / s5
    C[(1, 2, 1)] = np.transpose(B, (1, 0, 2)) / s5
    C[(2, 1, 1)] = B / s5
    C[(2, 2, 0)] = (np.eye(5) / s5)[:, :, None]
    T = np.einsum('aij,bjk,cki->abc', B, B, B)
    C[(2, 2, 2)] = T / np.linalg.norm(T)
    return C


CG = cg_np()
PATH_LIST_O2 = O2_UVW + O2_UVU


def support_pairs(path_ijk):
    d = {}
    for pi, (li, lj, lk) in enumerate(path_ijk):
        C = CG[(li, lj, lk)]
        for iloc in range(DIMS[li]):
            for jloc in range(DIMS[lj]):
                if np.any(np.abs(C[iloc, jloc, :]) > 1e-12):
                    d.setdefault(((li, iloc), (lj, jloc)), []).append((pi, iloc, jloc))
    return d


def build_mono_blocks_sym(path_ijk):
    d = support_pairs(path_ijk)
    blocks = {}
    for (I, J), lst in d.items():
        key = (min(I, J), max(I, J))
        swap = I > J
        for (pi, iloc, jloc) in lst:
            blocks.setdefault(key, []).append((pi, iloc, jloc, swap))
    return [(I, J, c) for (I, J), c in sorted(blocks.items())]


def build_mono_blocks(path_ijk):
    d = support_pairs(path_ijk)
    return [(I, J, [(pi, i, j, False) for (pi, i, j) in lst]) for (I, J), lst in sorted(d.items())]


def omega_for_block(path_ijk, weights, I, J, contribs, reg):
    """[1024 (u-major,v-fast), 576] interleaved outputs:
    col(g_out, reg, w) = g_out*64 + reg*32 + w."""
    Om = np.zeros((MUL * MUL, 2 * HID))
    for (pi, iloc, jloc, swap) in contribs:
        li, lj, lk = path_ijk[pi]
        W = weights[pi]
        C = CG[(li, lj, lk)]
        for kap in range(DIMS[lk]):
            c = C[iloc, jloc, kap]
            if abs(c) < 1e-12:
                continue
            gk = comp_ord(lk, kap)
            c0 = gk * 64 + reg * 32
            Wm = W if not swap else np.transpose(W, (1, 0, 2))
            Om[:, c0:c0 + 32] += c * Wm.reshape(MUL * MUL, MUL)
    return Om


# ---------------------------------------------------------------------------
# static plan
# ---------------------------------------------------------------------------

class Plan:
    pass


def _emissions(mask):
    """mask: [1024, 576] bool. Returns per kc: list of (c0, c1) col runs
    (gaptol 0 at 32-col-slot granularity, split at 512-wide)."""
    out = []
    for kc in range(8):
        sub = mask[kc * 128:(kc + 1) * 128]
        slots = [s for s in range(18) if np.any(sub[:, s * 32:(s + 1) * 32])]
        runs = []
        for s in slots:
            if runs and s == runs[-1][1]:
                runs[-1][1] = s + 1
            else:
                runs.append([s, s + 1])
        emis = []
        for (a, b) in runs:
            while (b - a) * 32 > 512:
                emis.append((a * 32, a * 32 + 512))
                a += 16
            emis.append((a * 32, b * 32))
        out.append(emis)
    return out


def build_plan():
    p = Plan()
    aa_blocks = build_mono_blocks_sym(PATHS_FULL + PATH_LIST_O2)
    qa_blocks = build_mono_blocks(PATHS_FULL)
    n3a = len(PATHS_FULL)
    ones_a = [np.ones((MUL, MUL, MUL)) for _ in PATHS_FULL]
    ones_o2 = [np.ones((MUL, MUL, MUL)) for _ in PATH_LIST_O2]

    p.aa = []
    for (I, J, contribs) in aa_blocks:
        cq = [(pi, i, j, s) for (pi, i, j, s) in contribs if pi < n3a]
        cm = [(pi - n3a, i, j, s) for (pi, i, j, s) in contribs if pi >= n3a]
        mask = np.zeros((1024, 576), bool)
        if cq:
            mask |= omega_for_block(PATHS_FULL, ones_a, I, J, cq, 0) != 0
        if cm:
            mask |= omega_for_block(PATH_LIST_O2, ones_o2, I, J, cm, 1) != 0
        p.aa.append((I, J, cq, cm, _emissions(mask)))
    p.qa = []
    for (I, J, contribs) in qa_blocks:
        mask = omega_for_block(PATHS_FULL, ones_a, I, J, contribs, 1) != 0
        p.qa.append((I, J, contribs, _emissions(mask)))

    # omega column offsets
    off = 0
    p.aa_emi = []
    for (I, J, cq, cm, em) in p.aa:
        bk = []
        for kc in range(8):
            lst = []
            for (c0, c1) in em[kc]:
                lst.append((c0, c1, off))
                off += c1 - c0
            bk.append(lst)
        p.aa_emi.append(bk)
    p.qa_emi = []
    for (I, J, contribs, em) in p.qa:
        bk = []
        for kc in range(8):
            lst = []
            for (c0, c1) in em[kc]:
                lst.append((c0, c1, off))
                off += c1 - c0
            bk.append(lst)
        p.qa_emi.append(bk)
    p.totc = off
    p.n_emi = sum(len(l) for bk in p.aa_emi + p.qa_emi for l in bk)

    # J-run groups for product ops: consecutive blocks with same I and
    # consecutive J ordinals, capped at MAX_JRUN
    def groups(blocks):
        gs = []
        for bi, blk in enumerate(blocks):
            I, J = blk[0], blk[1]
            gI = comp_ord(*I); gJ = comp_ord(*J)
            if (gs and gs[-1][0] == gI and gs[-1][1] + gs[-1][2] == gJ
                    and gs[-1][2] < MAX_JRUN):
                gs[-1][2] += 1
            else:
                gs.append([gI, gJ, 1, bi])
        return [(gI, gJ, n, b0) for (gI, gJ, n, b0) in gs]

    p.aa_groups = groups(p.aa)
    p.qa_groups = groups(p.qa)
    return p


def pack_omega(plan, Wfold):
    W3a = Wfold['o3a_w']; Wo2 = Wfold['o2_w']; W3b = Wfold['o3b_w']
    om = np.zeros((128, plan.totc), np.float32)
    for bi, (I, J, cq, cm, em) in enumerate(plan.aa):
        Om = np.zeros((MUL * MUL, 2 * HID))
        if cq:
            Om += omega_for_block(PATHS_FULL, W3a, I, J, cq, 0)
        if cm:
            Om += omega_for_block(PATH_LIST_O2, Wo2, I, J, cm, 1)
        for kc in range(8):
            for (c0, c1, off) in plan.aa_emi[bi][kc]:
                om[:, off:off + (c1 - c0)] = Om[kc * 128:(kc + 1) * 128, c0:c1]
    for bi, (I, J, contribs, em) in enumerate(plan.qa):
        Om = omega_for_block(PATHS_FULL, W3b, I, J, contribs, 1)
        for kc in range(8):
            for (c0, c1, off) in plan.qa_emi[bi][kc]:
                om[:, off:off + (c1 - c0)] = Om[kc * 128:(kc + 1) * 128, c0:c1]
    return om.astype(BF)


def fold_weights(inp):
    f8 = np.float64
    mix_w = inp['mix_w'].astype(f8); comb_w = inp['comb_w'].astype(f8)
    M = np.einsum('olux,olxw->oluw', mix_w, comb_w) / MUL
    W1eff = np.einsum('lux,lxw->luw', inp['lin_o1'].astype(f8), M[0]) / SQM
    o2_w = []
    for pp, (i, j, k) in enumerate(O2_UVW):
        o2_w.append(np.einsum('uvx,xw->uvw', inp['o2_uvw'][pp].astype(f8) / MUL, M[1][k]))
    for pp, (i, j, k) in enumerate(O2_UVU):
        o2_w.append(np.einsum('uv,uw->uvw', inp['o2_uvu'][pp].astype(f8), M[1][k]) / SQM)
    o3a_w = [inp['o3a_uvw'][pp].astype(f8) / MUL for pp in range(len(PATHS_FULL))]
    o3b_w = [np.einsum('uvx,xw->uvw', inp['o3b_uvw'][pp].astype(f8) / MUL, M[2][k])
             for pp, (i, j, k) in enumerate(PATHS_FULL)]
    aw = inp['a_w'].astype(f8).reshape(RHID, 3, MUL, MUL)
    ab = inp['a_b'].astype(f8).reshape(3, MUL, MUL)
    scale = np.array([1.0 / np.sqrt(d) for d in DIMS]) / SQM
    aw = aw * scale[None, :, None, None]
    ab = ab * scale[:, None, None]
    A2 = np.transpose(aw, (0, 2, 1, 3)).reshape(RHID * MUL, 3 * MUL)
    B2 = np.transpose(ab, (1, 0, 2)).reshape(MUL, 3 * MUL)
    # omc1: [32, 3*32]: per-l 32x32 order-1 linear (same for all i of that l)
    omc1 = np.zeros((32, 96))
    for l in range(3):
        omc1[:, l * 32:(l + 1) * 32] = W1eff[l]
    return dict(
        o3a_w=o3a_w, o2_w=o2_w, o3b_w=o3b_w,
        omc1=omc1, omself=inp['self_w'].astype(f8) / SQM,
        emb=inp['emb_w'].astype(f8) / SQM,
        A2=A2, B2=B2,
        r_w1=inp['r_w1'].astype(np.float32), r_b1=inp['r_b1'].astype(np.float32),
        r_w2=inp['r_w2'].astype(np.float32), r_b2=inp['r_b2'].astype(np.float32),
        r_w3=inp['r_w3'].astype(np.float32), r_b3=inp['r_b3'].astype(np.float32),
    )


def pack_edges(inp):
    src = np.asarray(inp['edge_index'][0]).astype(np.int64)
    dst = np.asarray(inp['edge_index'][1]).astype(np.int64)
    sh = np.asarray(inp['edge_sh'], dtype=np.float32)
    rad = np.asarray(inp['edge_radial_embedding'], dtype=np.float32)
    attr = np.asarray(inp['edge_attr'], dtype=np.float32)
    nf = np.asarray(inp['node_features'], dtype=np.float32)
    cnt = np.bincount(dst, minlength=N_NODES).astype(np.float32)
    rec_all = 1.0 / np.maximum(cnt, 1.0)
    order = np.argsort(dst, kind='stable')
    dst_s = dst[order]
    cores = []
    for c in range(N_CORES):
        lo = c * NODES_PER_CORE
        rinT = np.zeros((24, E_PAD), np.float32)
        nfsT = np.zeros((MUL, E_PAD), np.float32)
        sh9 = np.zeros((E_PAD, 9), np.float32)
        S = np.zeros((E_PAD, 128), BF)
        for w in range(N_WIN):
            nlo = lo + w * WIN
            nhi = min(lo + (w + 1) * WIN, lo + NODES_PER_CORE)
            a = np.searchsorted(dst_s, nlo); b = np.searchsorted(dst_s, nhi)
            idx = order[a:b]
            n = b - a
            assert n <= ESLOT, f"window overflow {n}"
            s = w * ESLOT
            rinT[:8, s:s + n] = rad[idx].T
            rinT[8:, s:s + n] = attr[idx].T
            nfsT[:, s:s + n] = nf[src[idx]].T
            sh9[s:s + n, :] = sh[idx]
            S[s + np.arange(n), (dst[idx] - nlo)] = BF(1.0)
        nfT = np.zeros((MUL, N_WIN * WIN), BF)
        nfT[:, :NODES_PER_CORE] = nf[lo:lo + NODES_PER_CORE].T.astype(BF)
        rec = np.ones((N_WIN * WIN, 1), np.float32)
        rec[:NODES_PER_CORE, 0] = rec_all[lo:lo + NODES_PER_CORE]
        cores.append(dict(rinT=rinT, nfsT=nfsT, sh9=sh9, S=S, nfT=nfT, rec=rec))
    return cores


def ref_from_kap(x_kap):
    out = np.empty_like(x_kap)
    for l, d in enumerate(DIMS):
        blk = x_kap[:, LOFF[l]:LOFF[l] + 32 * d].reshape(-1, d, 32)
        out[:, LOFF[l]:LOFF[l] + 32 * d] = np.transpose(blk, (0, 2, 1)).reshape(-1, 32 * d)
    return out


# ---------------------------------------------------------------------------
# device kernel
# ---------------------------------------------------------------------------

_NC_CACHE = {}
LAST_RESULT = None

# fraction of product work sent to gpsimd (tuned from profiles)
GP_ELEM_NS = 99.0e-3   # us per free-elem (effectively disable gpsimd)
VE_ELEM_NS = 0.52e-3
GP_OP_OH = 0.25
VE_OP_OH = 0.08


def build_nc(plan):
    import concourse.bass as bass
    import concourse.bacc as bacc
    import concourse.mybir as mybir
    import concourse.tile as tile

    f32 = mybir.dt.float32
    bf16 = mybir.dt.bfloat16
    AL = mybir.AluOpType
    AF = mybir.ActivationFunctionType

    nc = bacc.Bacc(None)
    P = 128

    # ---- dram parameters
    rinT_d = nc.declare_dram_parameter("rinT", [24, E_PAD], f32, isOutput=False)
    nfsT_d = nc.declare_dram_parameter("nfsT", [32, E_PAD], f32, isOutput=False)
    sh9_d = nc.declare_dram_parameter("sh9", [E_PAD, 9], f32, isOutput=False)
    S_d = nc.declare_dram_parameter("S", [E_PAD, 128], bf16, isOutput=False)
    nfT_d = nc.declare_dram_parameter("nfT", [32, N_WIN * WIN], bf16, isOutput=False)
    rec_d = nc.declare_dram_parameter("rec", [N_WIN * WIN, 1], f32, isOutput=False)
    omega_d = nc.declare_dram_parameter("omega", [P, plan.totc], bf16, isOutput=False)
    a2_d = nc.declare_dram_parameter("a2", [P, 16 * 96], bf16, isOutput=False)
    b2_d = nc.declare_dram_parameter("b2", [32, 96], bf16, isOutput=False)
    omc1_d = nc.declare_dram_parameter("omc1", [32, 96], bf16, isOutput=False)
    omself_d = nc.declare_dram_parameter("omself", [32, 32], bf16, isOutput=False)
    rw1_d = nc.declare_dram_parameter("rw1", [24, 64], f32, isOutput=False)
    rw2_d = nc.declare_dram_parameter("rw2", [64, 64], f32, isOutput=False)
    rw3_d = nc.declare_dram_parameter("rw3", [64, 64], f32, isOutput=False)
    rb1_d = nc.declare_dram_parameter("rb1", [64, 1], f32, isOutput=False)
    rb2_d = nc.declare_dram_parameter("rb2", [64, 1], f32, isOutput=False)
    emb_d = nc.declare_dram_parameter("emb", [32, 32], f32, isOutput=False)
    identb_d = nc.declare_dram_parameter("identb", [P, P], bf16, isOutput=False)
    selfull_d = nc.declare_dram_parameter("selfull", [P, 1024], bf16, isOutput=False)
    selr_d = nc.declare_dram_parameter("selr", [64, 2048], bf16, isOutput=False)
    repfull_d = nc.declare_dram_parameter("repfull", [P, P], bf16, isOutput=False)
    zer_d = nc.declare_dram_parameter("zer", [1, P], bf16, isOutput=False)
    zer2_d = nc.declare_dram_parameter("zer2", [1, 2 * HID], bf16, isOutput=False)
    out_d = nc.declare_dram_parameter("out", [N_WIN * WIN, HID], f32, isOutput=True)

    # engine schedule for product ops: greedy balance vector vs gpsimd
    def make_sched():
        ops = []
        for gi, (gI, gJ, nJ, b0) in enumerate(plan.aa_groups):
            ops.append(('aa', gi, nJ * 1024))
        for gi, (gI, gJ, nJ, b0) in enumerate(plan.qa_groups):
            ops.append(('qa', gi, nJ * 1024))
        for q in range(4):
            ops.append(('edge', q, 4096))
        v_t, g_t = 1.5, 0.0   # vector pre-loaded with msgs/evac budget
        sched = {}
        for (kind, idx, wdt) in ops:
            vc = wdt * VE_ELEM_NS + VE_OP_OH
            gc = wdt * GP_ELEM_NS + GP_OP_OH
            if g_t + gc < v_t + vc:
                sched[(kind, idx)] = 'gpsimd'; g_t += gc
            else:
                sched[(kind, idx)] = 'vector'; v_t += vc
        return sched

    sched = make_sched()

    from contextlib import ExitStack
    with tile.TileContext(nc) as tc, ExitStack() as es:
        cst = es.enter_context(tc.tile_pool(name="cst", bufs=1))
        sb2 = es.enter_context(tc.tile_pool(name="sb2", bufs=2))
        sb3 = es.enter_context(tc.tile_pool(name="sb3", bufs=2))
        uu_pool = es.enter_context(tc.tile_pool(name="uu", bufs=1))
        pt_pool = es.enter_context(tc.tile_pool(name="pt", bufs=9))
        ed_pool = es.enter_context(tc.tile_pool(name="ed", bufs=1))
        sb1 = es.enter_context(tc.tile_pool(name="sb1", bufs=1))
        ps_wps = es.enter_context(tc.tile_pool(name="pswps", bufs=1, space="PSUM"))
        ps_uub = es.enter_context(tc.tile_pool(name="psuub", bufs=2, space="PSUM"))
        ps_qm = es.enter_context(tc.tile_pool(name="psqm", bufs=1, space="PSUM"))
        ps_tp = es.enter_context(tc.tile_pool(name="pstp", bufs=1, space="PSUM"))
        ps_mlp = es.enter_context(tc.tile_pool(name="psmlp", bufs=1, space="PSUM"))
        ps_mx = es.enter_context(tc.tile_pool(name="psmx", bufs=1, space="PSUM"))

        # ---- constants
        omega = cst.tile([P, plan.totc], bf16)
        nc.sync.dma_start(out=omega[:], in_=omega_d[:])
        a2 = cst.tile([P, 16 * 96], bf16)
        nc.sync.dma_start(out=a2[:], in_=a2_d[:])
        b2 = cst.tile([32, 96], bf16); nc.sync.dma_start(out=b2[:], in_=b2_d[:])
        omc1 = cst.tile([32, 96], bf16); nc.sync.dma_start(out=omc1[:], in_=omc1_d[:])
        omself = cst.tile([32, 32], bf16); nc.sync.dma_start(out=omself[:], in_=omself_d[:])
        rw1 = cst.tile([24, 64], f32); nc.sync.dma_start(out=rw1[:], in_=rw1_d[:])
        rw2 = cst.tile([64, 64], f32); nc.sync.dma_start(out=rw2[:], in_=rw2_d[:])
        rw3 = cst.tile([64, 64], f32); nc.sync.dma_start(out=rw3[:], in_=rw3_d[:])
        rb1 = cst.tile([64, 1], f32); nc.sync.dma_start(out=rb1[:], in_=rb1_d[:])
        rb2 = cst.tile([64, 1], f32); nc.sync.dma_start(out=rb2[:], in_=rb2_d[:])
        emb = cst.tile([32, 32], f32); nc.sync.dma_start(out=emb[:], in_=emb_d[:])
        identb = cst.tile([P, P], bf16); nc.sync.dma_start(out=identb[:], in_=identb_d[:])
        selfull = cst.tile([P, 1024], bf16); nc.sync.dma_start(out=selfull[:], in_=selfull_d[:])
        selr = cst.tile([64, 2048], bf16); nc.sync.dma_start(out=selr[:], in_=selr_d[:])
        repfull = cst.tile([P, P], bf16); nc.sync.dma_start(out=repfull[:], in_=repfull_d[:])
        zer = cst.tile([1, P], bf16); nc.sync.dma_start(out=zer[:], in_=zer_d[:])
        zer2 = cst.tile([1, 2 * HID], bf16); nc.sync.dma_start(out=zer2[:], in_=zer2_d[:])
        nfT = cst.tile([32, N_WIN * WIN], bf16)
        nc.sync.dma_start(out=nfT[:], in_=nfT_d[:])

        def transpose3(x_bf, tag):
            """x_bf [128, 288] bf16 -> aT sbuf [128, 384] (chunk-major)."""
            tp = ps_tp.tile([P, 384], bf16, space="PSUM", tag="tp")
            nc.tensor.transpose(out=tp[:, 0:P], in_=x_bf[:, 0:P], identity=identb[:])
            nc.tensor.transpose(out=tp[:, P:2 * P], in_=x_bf[:, P:2 * P], identity=identb[:])
            nc.tensor.transpose(out=tp[0:32, 2 * P:3 * P], in_=x_bf[:, 2 * P:HID], identity=identb[:])
            xt = sb2.tile([P, 384], bf16, tag=tag + "sb")
            nc.scalar.copy(out=xt[:, 0:2 * P], in_=tp[:, 0:2 * P])
            nc.scalar.copy(out=xt[0:32, 2 * P:3 * P], in_=tp[0:32, 2 * P:3 * P])
            return xt

        NCC = [3, 2, 2, 2]   # comps per partition-row-group b: g = 4*cc + b <= 8

        def build_uu(aT, uu_tile, ev):
            """uu[32*u4+v, (g,kc,n)] = aT-val[f=32g+4kc+u4, node n] via SEL matmuls."""
            for b in range(4):
                ncc = NCC[b]
                for kc in range(8):
                    up = ps_uub.tile([P, 512], f32, space="PSUM", tag="uub")
                    nc.tensor.matmul(out=up[:, :ncc * P],
                                     lhsT=selfull[32 * b:32 * (b + 1), kc * P:(kc + 1) * P],
                                     rhs=aT[32 * b:32 * (b + 1), :ncc * P],
                                     start=True, stop=True, tile_position=(32 * b, 0))
                    dst = uu_tile[:].rearrange("p (g k n) -> p g k n", k=8, n=P)[:, b::4, kc, :]
                    src_ = up[:, :ncc * P].rearrange("p (c n) -> p c n", n=P)
                    nc.scalar.copy(out=dst, in_=src_)

        def build_v8(aT, v8_tile, ev):
            """v8[32*b+v, (g,n)] = aT-val[f=32g+v, node n] (mod-32 replication)."""
            for b in range(4):
                ncc = NCC[b]
                up = ps_uub.tile([P, 512], f32, space="PSUM", tag="uub")
                nc.tensor.matmul(out=up[:, :ncc * P],
                                 lhsT=repfull[32 * b:32 * (b + 1), :],
                                 rhs=aT[32 * b:32 * (b + 1), :ncc * P],
                                 start=True, stop=True, tile_position=(32 * b, 0))
                dst = v8_tile[:].rearrange("p (g n) -> p g n", n=P)[:, b::4, :]
                src_ = up[:, :ncc * P].rearrange("p (c n) -> p c n", n=P)
                nc.scalar.copy(out=dst, in_=src_)

        def emit_product_group(gi, groups, blocks, emi, uu, v8, qm, kind):
            (gI, gJ, nJ, b0) = groups[gi]
            wdt = nJ * 1024
            PT = pt_pool.tile([P, MAX_JRUN * 1024], bf16, tag="PT")
            eng = nc.gpsimd if sched[(kind, gi)] == 'gpsimd' else nc.vector
            eng.tensor_tensor(
                out=PT[:, :wdt].rearrange("p (j k n) -> p j k n", k=8, n=P),
                in0=uu[:, gI * 1024:(gI + 1) * 1024]
                    .rearrange("p (k n) -> p k n", n=P)[:, None, :, :]
                    .broadcast_to([P, nJ, 8, P]),
                in1=v8[:, gJ * P:(gJ + nJ) * P]
                    .rearrange("p (j n) -> p j n", n=P)[:, :, None, :]
                    .broadcast_to([P, nJ, 8, P]),
                op=AL.mult)
            for jl in range(nJ):
                bi = b0 + jl
                for kc in range(8):
                    for (c0, c1, off) in emi[bi][kc]:
                        nc.tensor.matmul(out=qm[:, c0:c1],
                                         lhsT=PT[:, jl * 1024 + kc * P: jl * 1024 + (kc + 1) * P],
                                         rhs=omega[:, off:off + (c1 - c0)],
                                         start=False, stop=False,
                                         skip_group_check=True)
        # per-window state for the software pipeline
        st = {}

        def emit_mlp_half(w, h):
            e0 = w * ESLOT
            if h == 0:
                rfT_t = sb1.tile([64, ESLOT], bf16, tag="rfT")
                hT_t = sb1.tile([32, ESLOT], bf16, tag="hT")
                vh_t = sb1.tile([P, ESLOT], bf16, tag="vh")
                wps_t = ps_wps.tile([P, HID], f32, space="PSUM", tag="wps")
                st[w] = dict(rfT=rfT_t, hT=hT_t, vh=vh_t, wps=wps_t, ev=[0])
            S = st[w]
            s = e0 + h * 512
            rin_h = sb2.tile([24, 512], f32, tag="rin")
            nc.sync.dma_start(out=rin_h[:], in_=rinT_d[:, s:s + 512])
            nfs_h = sb2.tile([32, 512], f32, tag="nfs")
            nc.sync.dma_start(out=nfs_h[:], in_=nfsT_d[:, s:s + 512])
            l1p = ps_mlp.tile([64, 512], f32, space="PSUM", tag="mlp")
            nc.tensor.matmul(out=l1p[:], lhsT=rw1[:], rhs=rin_h[:], start=True, stop=True)
            f1 = sb2.tile([64, 512], f32, tag="f")
            nc.scalar.activation(out=f1[:], in_=l1p[:], func=AF.Silu, bias=rb1[:], scale=1.0)
            l2p = ps_mlp.tile([64, 512], f32, space="PSUM", tag="mlp")
            nc.tensor.matmul(out=l2p[:], lhsT=rw2[:], rhs=f1[:], start=True, stop=True)
            f2 = sb2.tile([64, 512], f32, tag="f")
            nc.scalar.activation(out=f2[:], in_=l2p[:], func=AF.Silu, bias=rb2[:], scale=1.0)
            rfp = ps_mlp.tile([64, 512], f32, space="PSUM", tag="mlp")
            nc.tensor.matmul(out=rfp[:], lhsT=rw3[:], rhs=f2[:], start=True, stop=True)
            nc.scalar.copy(out=S['rfT'][:, h * 512:(h + 1) * 512], in_=rfp[:])
            hp = ps_mlp.tile([32, 512], f32, space="PSUM", tag="mlp")
            nc.tensor.matmul(out=hp[:], lhsT=emb[:], rhs=nfs_h[:], start=True, stop=True)
            nc.scalar.copy(out=S['hT'][:, h * 512:(h + 1) * 512], in_=hp[:])
            for b in range(4):
                nc.scalar.dma_start(out=S['vh'][32 * b:32 * (b + 1), h * 512:(h + 1) * 512],
                                    in_=S['hT'][:, h * 512:(h + 1) * 512])

        def emit_edge_quarter(w, q):
            e0 = w * ESLOT
            S = st[w]
            rfT, hT, vh, wps = S['rfT'], S['hT'], S['vh'], S['wps']
            uurf = ed_pool.tile([P, 4096], bf16, tag="uurf")
            for cp in range(8):
                up = ps_uub.tile([P, 512], f32, space="PSUM", tag="uub")
                for ci in range(2):
                    c = cp * 2 + ci
                    nc.tensor.matmul(out=up[:, ci * 256:(ci + 1) * 256],
                                     lhsT=selr[:, c * P:(c + 1) * P],
                                     rhs=rfT[:, q * 256:(q + 1) * 256],
                                     start=True, stop=True)
                nc.scalar.copy(out=uurf[:, cp * 512:(cp + 1) * 512], in_=up[:])
            mT = ed_pool.tile([P, 4096], bf16, tag="mT")
            eng = nc.gpsimd if sched[('edge', q)] == 'gpsimd' else nc.vector
            eng.tensor_tensor(
                out=mT[:].rearrange("p (c e) -> p c e", e=256),
                in0=uurf[:].rearrange("p (c e) -> p c e", e=256),
                in1=vh[:, q * 256:(q + 1) * 256][:, None, :].broadcast_to([P, 16, 256]),
                op=AL.mult)
            for tt in range(2):
                t = q * 2 + tt
                et = e0 + t * P
                mxp = ps_mx.tile([P, 96], f32, space="PSUM", tag="mx")
                for c in range(16):
                    nc.tensor.matmul(out=mxp[:], lhsT=mT[:, c * 256 + tt * P:c * 256 + (tt + 1) * P],
                                     rhs=a2[:, c * 96:(c + 1) * 96],
                                     start=(c == 0), stop=False)
                nc.tensor.matmul(out=mxp[:], lhsT=hT[:, t * P:(t + 1) * P], rhs=b2[:],
                                 start=False, stop=True)
                sh_t = sb3.tile([P, 9], f32, tag="sht")
                nc.sync.dma_start(out=sh_t[:], in_=sh9_d[et:et + P, :])
                msgs = sb2.tile([P, HID], bf16, tag="msgs")
                for l, d in enumerate(DIMS):
                    nc.vector.tensor_tensor(
                        out=msgs[:, LOFF[l]:LOFF[l] + 32 * d].rearrange("p (i u) -> p i u", u=32),
                        in0=sh_t[:, SOFF[l]:SOFF[l] + d][:, :, None].broadcast_to([P, d, 32]),
                        in1=mxp[:, l * 32:(l + 1) * 32][:, None, :].broadcast_to([P, d, 32]),
                        op=AL.mult)
                S_t = sb3.tile([P, P], bf16, tag="St")
                nc.sync.dma_start(out=S_t[:], in_=S_d[et:et + P, :])
                nc.tensor.matmul(out=wps[:], lhsT=S_t[:], rhs=msgs[:],
                                 start=(t == 0), stop=(t == ET_PER_WIN - 1))

        def emit_node_prefix(w):
            S = st[w]
            rec_t = sb2.tile([P, 1], f32, tag="rec")
            nc.sync.dma_start(out=rec_t[:], in_=rec_d[w * P:(w + 1) * P, :])
            a_bf = sb2.tile([P, HID], bf16, tag="abf")
            nc.vector.tensor_scalar_mul(out=a_bf[:], in0=S['wps'][:], scalar1=rec_t[:])
            aT = transpose3(a_bf, "at")
            uu = uu_pool.tile([P, NCOMP * 1024], bf16, tag="uu")
            build_uu(aT, uu, S['ev'])
            v8 = sb1.tile([P, NCOMP * P], bf16, tag="v8")
            build_v8(aT, v8, S['ev'])
            qm = ps_qm.tile([P, 2 * HID], f32, space="PSUM", tag="qm")
            nc.tensor.matmul(out=qm[:, 0:512], lhsT=zer[:], rhs=zer2[:, 0:512], start=True, stop=False, skip_group_check=True)
            nc.tensor.matmul(out=qm[:, 512:576], lhsT=zer[:], rhs=zer2[:, 512:576], start=True, stop=False, skip_group_check=True)
            S.update(uu=uu, v8=v8, qm=qm)

        def emit_node_qmid(w):
            S = st[w]
            q_bf = sb2.tile([P, HID], bf16, tag="qbf")
            nc.vector.tensor_copy(
                out=q_bf[:].rearrange("p (g c) -> p g c", c=32),
                in_=S['qm'][:].rearrange("p (g t c) -> p g t c", t=2, c=32)[:, :, 0, :])
            qT = transpose3(q_bf, "qt")
            uuq = uu_pool.tile([P, NCOMP * 1024], bf16, tag="uu")
            build_uu(qT, uuq, S['ev'])
            S.update(uuq=uuq)

        def emit_node_suffix(w):
            S = st[w]
            qm, v8 = S['qm'], S['v8']
            for g in range(NCOMP):
                l = COMP_L[g]
                nc.tensor.matmul(out=qm[:, g * 64 + 32:g * 64 + 64],
                                 lhsT=v8[0:32, g * P:(g + 1) * P],
                                 rhs=omc1[:, l * 32:(l + 1) * 32],
                                 start=False, stop=False, skip_group_check=True)
            nc.tensor.matmul(out=qm[:, 32:64], lhsT=nfT[:, w * P:(w + 1) * P],
                             rhs=omself[:], start=False, stop=True,
                             skip_group_check=True)
            out_sb = sb1.tile([P, HID], f32, tag="outsb")
            nc.scalar.copy(
                out=out_sb[:].rearrange("p (g c) -> p g c", c=32),
                in_=qm[:].rearrange("p (g t c) -> p g t c", t=2, c=32)[:, :, 1, :])
            nc.sync.dma_start(out=out_d[w * P:(w + 1) * P, :], in_=out_sb[:])
            del st[w]

        # ---------------- software-pipelined main loop ----------------
        # edge phase of window w+1 is interleaved into node phase of window w
        # so the PE queue always has ready work (keeps HAM warm).
        
        def node_events(w):
            ev = []
            ev.append(lambda w=w: emit_node_prefix(w))
            for gi in range(len(plan.aa_groups)):
                ev.append(lambda w=w, gi=gi: emit_product_group(gi, plan.aa_groups, plan.aa, plan.aa_emi, st[w]['uu'], st[w]['v8'], st[w]['qm'], 'aa'))
            ev.append(lambda w=w: emit_node_qmid(w))
            for gi in range(len(plan.qa_groups)):
                ev.append(lambda w=w, gi=gi: emit_product_group(gi, plan.qa_groups, plan.qa, plan.qa_emi, st[w]['uuq'], st[w]['v8'], st[w]['qm'], 'qa'))
            ev.append(lambda w=w: emit_node_suffix(w))
            return ev

        def edge_events(w):
            ev = [lambda w=w: emit_mlp_half(w, 0),
                  lambda w=w: emit_edge_quarter(w, 0),
                  lambda w=w: emit_edge_quarter(w, 1),
                  lambda w=w: emit_mlp_half(w, 1),
                  lambda w=w: emit_edge_quarter(w, 2),
                  lambda w=w: emit_edge_quarter(w, 3)]
            return ev

        # prologue: edge phase of window 0 runs alone
        for f in edge_events(0):
            f()
        for w in range(N_WIN):
            for f in node_events(w):
                f()
            if w + 1 < N_WIN:
                for f in edge_events(w + 1):
                    f()

    nc.finalize()
    return nc


def _get_nc(plan):
    if 'nc' not in _NC_CACHE:
        _NC_CACHE['nc'] = build_nc(plan)
    return _NC_CACHE['nc']


def kernel(**inputs):
    global LAST_RESULT
    from concourse.bass_utils import run_bass_kernel_spmd

    inp = {k: np.asarray(v) for k, v in inputs.items()}
    plan = build_plan()
    W = fold_weights(inp)
    om = pack_omega(plan, W)

    A2 = W['A2'].astype(np.float32)
    a2p = np.zeros((128, 16 * 96), np.float32)
    for c in range(16):
        a2p[:, c * 96:(c + 1) * 96] = A2[c * 128:(c + 1) * 128, :]
    # fold r_b3 into B2 (rf = f2 @ rw3; +b3 contribution is linear in h)
    B2 = W['B2'].astype(np.float64).copy()
    b3 = inp['r_b3'].astype(np.float64)
    for u in range(32):
        B2[u, :] += b3 @ A2[np.arange(RHID) * 32 + u, :].astype(np.float64)

    identb = np.eye(128, dtype=np.float32).astype(BF)
    self = None
    selfull = np.zeros((128, 1024), np.float32)
    for p in range(128):
        for kc in range(8):
            u4 = p % 32 - 4 * kc
            if 0 <= u4 < 4:
                selfull[p, kc * 128 + u4 * 32:kc * 128 + (u4 + 1) * 32] = 1.0
    selr = np.zeros((64, 2048), np.float32)
    for q in range(64):
        c, r4 = divmod(q, 4)
        selr[q, c * 128 + r4 * 32:c * 128 + (r4 + 1) * 32] = 1.0
    repfull = np.zeros((128, 128), np.float32)
    for p in range(128):
        for i in range(128):
            if i % 32 == p % 32:
                repfull[p, i] = 1.0

    shared = dict(
        omega=om,
        a2=a2p.astype(BF), b2=B2.astype(np.float32).astype(BF),
        omc1=W['omc1'].astype(np.float32).astype(BF),
        omself=W['omself'].astype(np.float32).astype(BF),
        rw1=W['r_w1'], rw2=W['r_w2'], rw3=W['r_w3'],
        rb1=W['r_b1'].reshape(64, 1), rb2=W['r_b2'].reshape(64, 1),
        emb=W['emb'].astype(np.float32),
        identb=identb,
        selfull=selfull.astype(BF), selr=selr.astype(BF), repfull=repfull.astype(BF),
        zer=np.zeros((1, 128), BF), zer2=np.zeros((1, 2 * HID), BF),
    )
    cores = pack_edges(inp)
    in_maps = []
    for c in range(N_CORES):
        m = dict(shared)
        m.update(rinT=cores[c]['rinT'], nfsT=cores[c]['nfsT'],
                 sh9=cores[c]['sh9'], S=cores[c]['S'], nfT=cores[c]['nfT'],
                 rec=cores[c]['rec'])
        in_maps.append(m)

    nc = _get_nc(plan)
    res = run_bass_kernel_spmd(nc, in_maps, core_ids=list(range(N_CORES)))
    LAST_RESULT = res
    outs = [res.results[c]['out'][:NODES_PER_CORE] for c in range(N_CORES)]
    out_kap = np.concatenate(outs, axis=0).astype(np.float32)
    return ref_from_kap(out_kap)


if __name__ == "__main__":
    plan = build_plan()
    print(f"aa blocks: {len(plan.aa)}  qa blocks: {len(plan.qa)}")
    print(f"aa groups: {len(plan.aa_groups)}  qa groups: {len(plan.qa_groups)}")
    print(f"omega cols: {plan.totc}  ({plan.totc * 128 * 2 / 1e6:.1f} MB bf16)")
    print(f"emissions per node-tile: {plan.n_emi}")
